# revision 1
# baseline (speedup 1.0000x reference)
"""Trainium2 8-core Bass kernel for a dual cross-attention transformer block.

Sharding: tensor-parallel attention heads (16 heads -> 2/core) for both the
x-side and y-side attention; token-parallel FFN (4096 tokens -> 512/core) with
full FFN weights replicated.  Comms: AllGather of the RMS-normed, transposed
activations at the start; ReduceScatter of the Wo partial sums (token-sharded)
per side.  All activations live feature-on-partition ("transposed") so biases
and norm weights are per-partition scalars.  Compute dtype bf16, fp32 PSUM.

v2: V kept in SBUF (no DRAM roundtrip); attention/Wo/FFN pools hoisted across
sides so the scheduler can overlap Wo(x) with attention(y) and Wo(y)/RS(y)
with FFN(x); softmax kept ACT-exp-only (aux copies moved to DVE/Pool); PSUM
bank budget: attention 6 + Wo 2 = 8, FFN 6 + Wo 2 = 8.
"""

import math

import numpy as np
import ml_dtypes

import concourse.bass as bass
import concourse.tile as tile
from concourse import mybir, bacc
from concourse.bass_utils import run_bass_kernel_spmd

B, S, D, H = 2, 2048, 2048, 16
HD = D // H            # 128
HID = 5632
EPS = 1e-5
NC = 8                 # cores
HPC = H // NC          # 2 heads per core
T = B * S              # 4096 tokens
TPC = T // NC          # 512 tokens per core
KCH = D // 128         # 16 contraction chunks over D
JCH = HID // 128       # 44 chunks over HID
QBLK = 256             # token block for QKV projections
NQB = T // QBLK        # 16
BF = ml_dtypes.bfloat16
E4 = ml_dtypes.float8_e4m3
F32 = mybir.dt.float32
BF16 = mybir.dt.bfloat16
FP8 = mybir.dt.float8e4
import os as _os
USE_FP8 = _os.environ.get("BASS_FP8", "1") == "1"
FSC = 16.0

_CACHE = {}


def _rope_perm():
    # [evens, odds]: puts x1 in partitions 0:64, x2 in 64:128 of Q^T/K^T
    return np.concatenate([np.arange(0, 128, 2), np.arange(1, 128, 2)])


def build_nc(sim_local=False, reps=1, fp8=None):
    if fp8 is None:
        fp8 = USE_FP8
    nc = bacc.Bacc("TRN2", target_bir_lowering=False, debug=False,
                   num_devices=1 if sim_local else NC)

    AL = mybir.AluOpType
    AF = mybir.ActivationFunctionType

    # ---------------- external parameters ----------------
    xT = {s: nc.declare_dram_parameter(f"{s}T", [D, TPC], F32, isOutput=False)
          for s in ("x", "y")}
    # rope tables, stacked [c;c] and [-s;s]; q-variant pre-scaled by 1/sqrt(HD)
    tabs = {n: nc.declare_dram_parameter(n, [128, S], F32, isOutput=False)
            for n in ("cs_q", "sn_q", "cs_k", "sn_k")}
    wq, wk, wv, wo = {}, {}, {}, {}
    bq, bqs, bk, bks, bv, bo = {}, {}, {}, {}, {}, {}
    w1, w3, w2, b1, b3, b2, fnw = {}, {}, {}, {}, {}, {}, {}
    for s in ("x", "y"):
        wq[s] = nc.declare_dram_parameter(f"wq_{s}", [HPC, 128, KCH, 128], BF16, isOutput=False)
        wk[s] = nc.declare_dram_parameter(f"wk_{s}", [HPC, 128, KCH, 128], BF16, isOutput=False)
        wv[s] = nc.declare_dram_parameter(f"wv_{s}", [128, KCH, HPC * 128], BF16, isOutput=False)
        wo[s] = nc.declare_dram_parameter(f"wo_{s}", [HPC, 128, KCH, 128], BF16, isOutput=False)
        bq[s] = nc.declare_dram_parameter(f"bq_{s}", [HPC, 128], F32, isOutput=False)
        bqs[s] = nc.declare_dram_parameter(f"bqs_{s}", [HPC, 128], F32, isOutput=False)
        bk[s] = nc.declare_dram_parameter(f"bk_{s}", [HPC, 128], F32, isOutput=False)
        bks[s] = nc.declare_dram_parameter(f"bks_{s}", [HPC, 128], F32, isOutput=False)
        bv[s] = nc.declare_dram_parameter(f"bv_{s}", [HPC * 128], F32, isOutput=False)
        bo[s] = nc.declare_dram_parameter(f"bo_{s}", [KCH, 128], F32, isOutput=False)
        wdt = FP8 if fp8 else BF16
        w1[s] = nc.declare_dram_parameter(f"w1_{s}", [JCH, 128, KCH, 128], wdt, isOutput=False)
        w3[s] = nc.declare_dram_parameter(f"w3_{s}", [JCH, 128, KCH, 128], wdt, isOutput=False)
        w2[s] = nc.declare_dram_parameter(f"w2_{s}", [KCH, 128, JCH, 128], wdt, isOutput=False)
        b1[s] = nc.declare_dram_parameter(f"b1_{s}", [JCH, 128], F32, isOutput=False)
        b3[s] = nc.declare_dram_parameter(f"b3_{s}", [JCH, 128], F32, isOutput=False)
        b2[s] = nc.declare_dram_parameter(f"b2_{s}", [KCH, 128], F32, isOutput=False)
        fnw[s] = nc.declare_dram_parameter(f"fnw_{s}", [KCH, 128], F32, isOutput=False)
    anw = nc.declare_dram_parameter("anw", [KCH, 128], F32, isOutput=False)
    out_ext = nc.declare_dram_parameter("out", [2, D, TPC], F32, isOutput=True)

    # ---------------- internal DRAM ----------------
    ag_in = {(s, h): nc.dram_tensor(f"ag_in_{s}{h}", [D, TPC // 2], BF16)
             for s in ("x", "y") for h in (0, 1)}
    ag_out = {(s, h): nc.dram_tensor(f"ag_out_{s}{h}", [NC, D, TPC // 2], BF16,
                                     addr_space="Shared")
              for s in ("x", "y") for h in (0, 1)}
    rs_in = {s: nc.dram_tensor(f"rs_in_{s}", [NC, D, TPC], BF16) for s in ("x", "y")}
    rs_out = {s: nc.dram_tensor(f"rs_out_{s}", [D, TPC], BF16)
              for s in ("x", "y")}

    rg = [list(range(NC))]

    from contextlib import ExitStack
    with tile.TileContext(nc) as tc:
        with ExitStack() as es:
            const = es.enter_context(tc.tile_pool(name="const", bufs=1))
            ones_bf = const.tile([128, 1], BF16)
            nc.vector.memset(ones_bf, 1.0)
            ones_f = const.tile([128, 1], F32)
            nc.vector.memset(ones_f, 1.0)
            ones_row = const.tile([1, 128], F32)
            nc.vector.memset(ones_row, 1.0)
            sc_row = const.tile([1, 128], F32)
            nc.vector.memset(sc_row, 1.0)
            eps_sb = const.tile([128, 1], F32)
            nc.vector.memset(eps_sb, EPS)
            anw_sb = const.tile([128, KCH], F32)
            nc.sync.dma_start(out=anw_sb, in_=anw.rearrange("k p -> p k"))
            fnw_sb, bo_sb, b2_sb = {}, {}, {}
            bq_sb, bqs_sb, bk_sb, bks_sb, bv_sb = {}, {}, {}, {}, {}
            b1_sb, b3_sb = {}, {}
            for s in ("x", "y"):
                fnw_sb[s] = const.tile([128, KCH], F32, name=f"fnw{s}", tag=f"fnw{s}")
                nc.sync.dma_start(out=fnw_sb[s], in_=fnw[s].rearrange("k p -> p k"))
                bo_sb[s] = const.tile([128, KCH], F32, name=f"bo{s}", tag=f"bo{s}")
                nc.sync.dma_start(out=bo_sb[s], in_=bo[s].rearrange("k p -> p k"))
                b2_sb[s] = const.tile([128, KCH], F32, name=f"b2{s}", tag=f"b2{s}")
                nc.sync.dma_start(out=b2_sb[s], in_=b2[s].rearrange("k p -> p k"))
                b1_sb[s] = const.tile([128, JCH], F32, name=f"b1{s}", tag=f"b1{s}")
                nc.sync.dma_start(out=b1_sb[s], in_=b1[s].rearrange("k p -> p k"))
                b3_sb[s] = const.tile([128, JCH], F32, name=f"b3{s}", tag=f"b3{s}")
                nc.sync.dma_start(out=b3_sb[s], in_=b3[s].rearrange("k p -> p k"))
                bq_sb[s] = const.tile([128, HPC], F32, name=f"bq{s}", tag=f"bq{s}")
                nc.sync.dma_start(out=bq_sb[s], in_=bq[s].rearrange("h p -> p h"))
                bqs_sb[s] = const.tile([128, HPC], F32, name=f"bqs{s}", tag=f"bqs{s}")
                nc.sync.dma_start(out=bqs_sb[s], in_=bqs[s].rearrange("h p -> p h"))
                bk_sb[s] = const.tile([128, HPC], F32, name=f"bk{s}", tag=f"bk{s}")
                nc.sync.dma_start(out=bk_sb[s], in_=bk[s].rearrange("h p -> p h"))
                bks_sb[s] = const.tile([128, HPC], F32, name=f"bks{s}", tag=f"bks{s}")
                nc.sync.dma_start(out=bks_sb[s], in_=bks[s].rearrange("h p -> p h"))
                bv_sb[s] = const.tile([128, HPC * 128], F32, name=f"bv{s}", tag=f"bv{s}")
                nc.sync.dma_start(out=bv_sb[s],
                                  in_=bv[s][None, :].to_broadcast([128, HPC * 128]))

            for _rep in range(reps):
                # ---------- phase 1: rms-norm own token shard, write ag_in ----------
                with tc.tile_pool(name="nrm", bufs=2) as nrm, \
                     tc.tile_pool(name="nrm_ps", bufs=2, space="PSUM") as nrm_ps:
                    for s in ("x", "y"):
                        xt_sb = []
                        ms_ps = nrm_ps.tile([1, TPC], F32, name="ms", tag="ms")
                        for kc in range(KCH):
                            t = nrm.tile([128, TPC], F32, name="xt", tag="xt", bufs=18)
                            nc.sync.dma_start(out=t, in_=xT[s][kc * 128:(kc + 1) * 128, :])
                            xt_sb.append(t)
                            sq = nrm.tile([128, TPC], F32, name="sq", tag="sq")
                            nc.scalar.activation(out=sq, in_=t, func=AF.Square)
                            nc.tensor.matmul(ms_ps[:, 0:TPC], ones_f, sq,
                                             start=(kc == 0), stop=(kc == KCH - 1))
                        sd = nrm.tile([1, TPC], F32, name="sd", tag="sd")
                        nc.scalar.activation(out=sd, in_=ms_ps, func=AF.Sqrt,
                                             bias=eps_sb[0:1, :], scale=1.0 / D)
                        rec = nrm.tile([1, TPC], F32, name="rec", tag="rec")
                        nc.vector.reciprocal(out=rec, in_=sd)
                        rb_ps = nrm_ps.tile([128, TPC], F32, name="rb", tag="rb")
                        nc.tensor.matmul(rb_ps, sc_row, rec, start=True, stop=True)
                        rb = nrm.tile([128, TPC], F32, name="rbs", tag="rbs")
                        nc.scalar.copy(out=rb, in_=rb_ps)
                        for kc in range(KCH):
                            nt = nrm.tile([128, TPC], BF16, name="nt", tag="nt")
                            nc.vector.scalar_tensor_tensor(
                                out=nt, in0=xt_sb[kc], scalar=anw_sb[:, kc:kc + 1],
                                in1=rb, op0=AL.mult, op1=AL.mult)
                            for h in (0, 1):
                                nc.sync.dma_start(
                                    out=ag_in[(s, h)][kc * 128:(kc + 1) * 128, :],
                                    in_=nt[:, h * (TPC // 2):(h + 1) * (TPC // 2)])

                # ---------- phase 2: all-gather normalized activations ----------
                for h in (0, 1):
                    for s in ("x", "y"):
                        if sim_local:
                            for g in range(NC):
                                nc.sync.dma_start(out=ag_out[(s, h)][g],
                                                  in_=ag_in[(s, h)][:])
                        else:
                            nc.gpsimd.collective_compute(
                                "AllGather", AL.bypass, replica_groups=rg,
                                ins=[ag_in[(s, h)][:]], outs=[ag_out[(s, h)][:]])

                # ---------- phase 3: QKV projections (both sides) ----------
                p35_es = ExitStack()
                qkt = p35_es.enter_context(tc.tile_pool(name="qkt", bufs=1))
                qt_sb, kt_sb = {}, {}
                for s in ("x", "y"):
                    for h in range(HPC):
                        qt_sb[(s, h)] = qkt.tile([128, T], BF16, name=f"qt{s}{h}", tag=f"qt{s}{h}")
                        kt_sb[(s, h)] = qkt.tile([128, T], BF16, name=f"kt{s}{h}", tag=f"kt{s}{h}")
                # V kept resident in SBUF: [tok%128, tok//128, HPC*HD]
                v_sb = {}
                for s in ("x", "y"):
                    v_sb[s] = qkt.tile([128, T // 128, HPC * 128], BF16,
                                       name=f"v{s}", tag=f"v{s}")

                with tc.tile_pool(name="qkvw", bufs=1) as qkvw:
                    wq_sb, wk_sb, wv_sb = {}, {}, {}
                    for s in ("x", "y"):
                        for h in range(HPC):
                            wq_sb[(s, h)] = qkvw.tile([128, KCH, 128], BF16, name=f"wq{s}{h}", tag=f"wq{s}{h}")
                            nc.sync.dma_start(out=wq_sb[(s, h)], in_=wq[s][h])
                            wk_sb[(s, h)] = qkvw.tile([128, KCH, 128], BF16, name=f"wk{s}{h}", tag=f"wk{s}{h}")
                            nc.sync.dma_start(out=wk_sb[(s, h)], in_=wk[s][h])
                        wv_sb[s] = qkvw.tile([128, KCH, HPC * 128], BF16, name=f"wv{s}", tag=f"wv{s}")
                        nc.sync.dma_start(out=wv_sb[s], in_=wv[s][:])

                    with tc.tile_pool(name="acts", bufs=2) as acts, \
                         tc.tile_pool(name="ropet", bufs=2) as ropet, \
                         tc.tile_pool(name="ropes", bufs=3) as ropes, \
                         tc.tile_pool(name="qkv_ps", bufs=2, space="PSUM") as qkv_ps:
                        for qb in range(NQB):
                            g, half = qb // 2, qb % 2
                            pos = (qb * QBLK) % S
                            a_sb = {}
                            for s in ("x", "y"):
                                a = acts.tile([128, KCH, QBLK], BF16, name=f"a{s}",
                                              tag="a", bufs=3)
                                nc.sync.dma_start(
                                    out=a, in_=ag_out[(s, half)][g].rearrange(
                                        "(k p) t -> p k t", p=128))
                                a_sb[s] = a
                            tb = {}
                            for n in ("cs_q", "sn_q", "cs_k", "sn_k"):
                                tt = ropet.tile([128, QBLK], F32, tag=n)
                                nc.sync.dma_start(out=tt, in_=tabs[n][:, pos:pos + QBLK])
                                tb[n] = tt
                            for s in ("x", "y"):
                                src_q = a_sb["y" if s == "x" else "x"]  # queries from hidden
                                src_kv = a_sb[s]                        # keys/values from ctx
                                for h in range(HPC):
                                    for proj, wsb, bsb, bssb, cs_t, sn_t, dst in (
                                        ("q", wq_sb[(s, h)], bq_sb[s], bqs_sb[s],
                                         tb["cs_q"], tb["sn_q"], qt_sb[(s, h)]),
                                        ("k", wk_sb[(s, h)], bk_sb[s], bks_sb[s],
                                         tb["cs_k"], tb["sn_k"], kt_sb[(s, h)]),
                                    ):
                                        src = src_q if proj == "q" else src_kv
                                        ps = qkv_ps.tile([128, QBLK], F32, name="qk", tag="qk")
                                        for kc in range(KCH):
                                            nc.tensor.matmul(
                                                ps, wsb[:, kc, :], src[:, kc, :],
                                                start=(kc == 0), stop=(kc == KCH - 1))
                                        # rope: copy PSUM->SBUF, swap halves via
                                        # DMA, then (q+b)*cs + (qsw+bsw)*sn
                                        qs = ropes.tile([128, QBLK], F32, name="qs", tag="qs")
                                        nc.scalar.copy(out=qs, in_=ps)
                                        qsw = ropes.tile([128, QBLK], F32, name="qsw", tag="qsw")
                                        nc.sync.dma_start(out=qsw[0:64, :], in_=qs[64:128, :])
                                        nc.sync.dma_start(out=qsw[64:128, :], in_=qs[0:64, :])
                                        t1 = ropes.tile([128, QBLK], F32, name="t1", tag="t1")
                                        nc.vector.scalar_tensor_tensor(
                                            out=t1, in0=qs, scalar=bsb[:, h:h + 1],
                                            in1=cs_t, op0=AL.add, op1=AL.mult)
                                        t2 = ropes.tile([128, QBLK], F32, name="t2", tag="t2")
                                        nc.vector.scalar_tensor_tensor(
                                            out=t2, in0=qsw, scalar=bssb[:, h:h + 1],
                                            in1=sn_t, op0=AL.add, op1=AL.mult)
                                        nc.vector.tensor_add(
                                            dst[:, qb * QBLK:(qb + 1) * QBLK], t1, t2)
                                # V in natural [token, hd] layout -> SBUF resident
                                for tk in range(QBLK // 128):
                                    vps = qkv_ps.tile([128, HPC * 128], F32, name="v", tag="v")
                                    for kc in range(KCH):
                                        nc.tensor.matmul(
                                            vps, src_kv[:, kc, tk * 128:(tk + 1) * 128],
                                            wv_sb[s][:, kc, :],
                                            start=(kc == 0), stop=(kc == KCH - 1))
                                    nc.vector.tensor_add(
                                        v_sb[s][:, qb * 2 + tk, :], vps, bv_sb[s])

                # ---------- phase 4/5: attention + Wo + RS, pools hoisted ----------
                oT_pool = p35_es.enter_context(tc.tile_pool(name="oT", bufs=1))
                att_pt = p35_es.enter_context(tc.tile_pool(name="att_pt", bufs=2))
                att_sb = p35_es.enter_context(tc.tile_pool(name="att_sb", bufs=2))
                wo_pool = p35_es.enter_context(tc.tile_pool(name="wo_w", bufs=1))
                wo_s = p35_es.enter_context(tc.tile_pool(name="wo_s", bufs=4))
                att_ps = p35_es.enter_context(
                    tc.tile_pool(name="att_ps", bufs=2, space="PSUM"))
                att_po = p35_es.enter_context(
                    tc.tile_pool(name="att_po", bufs=1, space="PSUM"))
                wo_ps_pool = p35_es.enter_context(
                    tc.tile_pool(name="wo_ps", bufs=2, space="PSUM"))

                oT = {}
                wo_sb = {}
                for s in ("x", "y"):
                    for h in range(HPC):
                        oT[(s, h)] = oT_pool.tile([128, T], BF16,
                                                  name=f"o{s}{h}", tag=f"o{s}{h}")
                        wo_sb[(s, h)] = wo_pool.tile([128, KCH, 128], BF16,
                                                     name=f"wo{s}{h}", tag=f"wo{s}{h}")
                        nc.sync.dma_start(out=wo_sb[(s, h)], in_=wo[s][h])

                for s in ("x", "y"):
                    for b in range(B):
                        for blk in range(S // 1024):
                            tq0 = b * S + blk * 1024
                            for h in range(HPC):
                                ops = [att_po.tile([128, 512], F32, name="o0", tag="o0"),
                                       att_po.tile([128, 512], F32, name="o1", tag="o1")]
                                stack = []  # eager pairwise sum tree of exp chunks
                                for tkc in range(16):
                                    sps = att_ps.tile([128, 1024], F32, name="s", tag="s")
                                    for i in (0, 1):
                                        nc.tensor.matmul(
                                            sps[:, i * 512:(i + 1) * 512],
                                            kt_sb[(s, h)][:, b * S + tkc * 128:
                                                          b * S + (tkc + 1) * 128],
                                            qt_sb[(s, h)][:, tq0 + i * 512:
                                                          tq0 + (i + 1) * 512],
                                            start=True, stop=True)
                                    p = att_pt.tile([128, 1024], BF16, name="pt",
                                                    tag="pt", bufs=6)
                                    nc.scalar.activation(out=p, in_=sps, func=AF.Exp)
                                    for i in (0, 1):
                                        nc.tensor.matmul(
                                            ops[i],
                                            v_sb[s][:, b * (S // 128) + tkc,
                                                    h * 128:(h + 1) * 128],
                                            p[:, i * 512:(i + 1) * 512],
                                            start=(tkc == 0), stop=(tkc == 15))
                                    cur = (0, p)
                                    while stack and stack[-1][0] == cur[0]:
                                        lvl, other = stack.pop()
                                        dsum = att_pt.tile(
                                            [128, 1024], BF16, name="ds",
                                            tag=f"ds{lvl}", bufs=2)
                                        nc.vector.tensor_add(dsum, other, cur[1])
                                        cur = (lvl + 1, dsum)
                                    stack.append(cur)
                                lvl_final = stack[-1][1]
                                # denominator + reciprocal-broadcast borrow the
                                # Wo PSUM tag (1-bank tiles, released fast) so
                                # the scores tag never stalls across blocks
                                rec = att_sb.tile([1, 1024], F32, name="rec", tag="rec")
                                for i in (0, 1):
                                    dn = wo_ps_pool.tile([1, 512], F32,
                                                         name="dn", tag="wops")
                                    nc.tensor.matmul(
                                        dn, ones_bf,
                                        lvl_final[:, i * 512:(i + 1) * 512],
                                        start=True, stop=True)
                                    nc.vector.reciprocal(
                                        out=rec[:, i * 512:(i + 1) * 512], in_=dn)
                                rbs = att_sb.tile([128, 1024], F32, name="rbs", tag="rbs")
                                for i in (0, 1):
                                    rb = wo_ps_pool.tile([128, 512], F32,
                                                         name="rbb", tag="wops")
                                    nc.tensor.matmul(
                                        rb, ones_row, rec[:, i * 512:(i + 1) * 512],
                                        start=True, stop=True)
                                    nc.any.tensor_copy(
                                        out=rbs[:, i * 512:(i + 1) * 512], in_=rb)
                                for i in (0, 1):
                                    nc.vector.tensor_mul(
                                        oT[(s, h)][:, tq0 + i * 512:tq0 + (i + 1) * 512],
                                        ops[i], rbs[:, i * 512:(i + 1) * 512])
                            # Wo for the two 512-token groups just finished
                            # (both heads done) -> rs_in; fills PE/ACT/DVE gaps
                            for g in (tq0 // 512, tq0 // 512 + 1):
                                for kc in range(KCH):
                                    wps = wo_ps_pool.tile([128, 512], F32,
                                                          name="wops", tag="wops")
                                    for h in range(HPC):
                                        nc.tensor.matmul(
                                            wps, wo_sb[(s, h)][:, kc, :],
                                            oT[(s, h)][:, g * 512:(g + 1) * 512],
                                            start=(h == 0), stop=(h == HPC - 1))
                                    st = wo_s.tile([128, 512], BF16, name="st", tag="st")
                                    nc.any.tensor_copy(out=st, in_=wps)
                                    nc.sync.dma_start(
                                        out=rs_in[s][g, kc * 128:(kc + 1) * 128, :],
                                        in_=st)
                    if sim_local:
                        for g in range(NC):
                            nc.sync.dma_start(out=rs_out[s][:], in_=rs_in[s][g])
                    else:
                        nc.gpsimd.collective_compute(
                            "ReduceScatter", AL.add, replica_groups=rg,
                            ins=[rs_in[s][:]], outs=[rs_out[s][:]])
                p35_es.close()

                # ---------- phase 6: FFN + residual + final norm per side ----------
                with tc.tile_pool(name="ffn_h", bufs=1) as ffn_h, \
                     tc.tile_pool(name="ffn_g", bufs=1) as ffn_g, \
                     tc.tile_pool(name="ffn_w", bufs=3) as ffn_w, \
                     tc.tile_pool(name="ffn_t", bufs=2) as ffn_t, \
                     tc.tile_pool(name="ffn_ps", bufs=2, space="PSUM") as ffn_ps, \
                     tc.tile_pool(name="ffn_ps1", bufs=1, space="PSUM") as ffn_ps1:
                    DR = mybir.MatmulPerfMode.DoubleRow
                    for s in ("x", "y"):
                        hT = []
                        h8 = [ffn_h.tile([128, 2, TPC], FP8, name=f"h8{s}_{i}",
                                         tag=f"h8{s}_{i}") for i in range(KCH // 2)] \
                            if fp8 else None
                        for kc in range(KCH):
                            raw = ffn_t.tile([128, TPC], BF16, name="raw", tag="raw")
                            nc.sync.dma_start(out=raw,
                                              in_=rs_out[s][kc * 128:(kc + 1) * 128, :])
                            if fp8:
                                nc.vector.tensor_scalar(
                                    h8[kc // 2][:, kc % 2, :], raw,
                                    bo_sb[s][:, kc:kc + 1], FSC,
                                    op0=AL.add, op1=AL.mult)
                            else:
                                ht = ffn_h.tile([128, TPC], BF16, name=f"h{s}{kc}",
                                                tag=f"h{s}{kc}")
                                nc.vector.tensor_scalar_add(
                                    ht, raw, bo_sb[s][:, kc:kc + 1])
                                hT.append(ht)
                        g_sb = []
                        for jc in range(JCH):
                            wdt2 = FP8 if fp8 else BF16
                            wsh = [128, KCH // 2, 2, 128] if fp8 else [128, KCH, 128]
                            w1t = ffn_w.tile(wsh, wdt2, name="w1", tag="w1")
                            nc.sync.dma_start(out=w1t, in_=w1[s][jc])
                            w3t = ffn_w.tile(wsh, wdt2, name="w3", tag="w3")
                            nc.sync.dma_start(out=w3t, in_=w3[s][jc])
                            z1 = ffn_ps.tile([128, TPC], F32, name="z1", tag="z", bufs=3)
                            z3 = ffn_ps.tile([128, TPC], F32, name="z3", tag="z", bufs=3)
                            if fp8:
                                for kp in range(KCH // 2):
                                    nc.tensor.matmul(z1, w1t[:, kp, :, :], h8[kp],
                                                     start=(kp == 0),
                                                     stop=(kp == KCH // 2 - 1),
                                                     perf_mode=DR)
                                for kp in range(KCH // 2):
                                    nc.tensor.matmul(z3, w3t[:, kp, :, :], h8[kp],
                                                     start=(kp == 0),
                                                     stop=(kp == KCH // 2 - 1),
                                                     perf_mode=DR)
                            else:
                                for kc in range(KCH):
                                    nc.tensor.matmul(z1, w1t[:, kc, :], hT[kc],
                                                     start=(kc == 0), stop=(kc == KCH - 1))
                                for kc in range(KCH):
                                    nc.tensor.matmul(z3, w3t[:, kc, :], hT[kc],
                                                     start=(kc == 0), stop=(kc == KCH - 1))
                            sz = ffn_t.tile([128, TPC], F32, name="sz", tag="sz")
                            nc.scalar.activation(out=sz, in_=z1, func=AF.Silu,
                                                 bias=b1_sb[s][:, jc:jc + 1],
                                                 scale=1.0 / (FSC * FSC) if fp8 else 1.0)
                            gt = ffn_g.tile([128, TPC], BF16, name="gt",
                                            tag="gt" if fp8 else f"g{s}{jc}",
                                            bufs=2 if fp8 else None)
                            nc.vector.scalar_tensor_tensor(
                                out=gt, in0=z3, scalar=b3_sb[s][:, jc:jc + 1], in1=sz,
                                op0=AL.add, op1=AL.mult)
                            if fp8:
                                if jc % 2 == 0:
                                    g8 = ffn_g.tile([128, 2, TPC], FP8,
                                                    name=f"g8{s}_{jc // 2}",
                                                    tag=f"g8{s}_{jc // 2}")
                                    g_sb.append(g8)
                                nc.scalar.mul(out=g_sb[jc // 2][:, jc % 2, :],
                                              in_=gt, mul=1.0 / FSC)
                            else:
                                g_sb.append(gt)
                        # W2 pass + residual + stats
                        ffr = []
                        ms_ps = ffn_ps1.tile([1, TPC], F32, name="ms", tag="aux")
                        for kc in range(KCH):
                            wsh2 = [128, JCH // 2, 2, 128] if fp8 else [128, JCH, 128]
                            w2t = ffn_w.tile(wsh2, FP8 if fp8 else BF16,
                                             name="w2", tag="w2", bufs=2)
                            nc.sync.dma_start(out=w2t, in_=w2[s][kc])
                            ff = ffn_ps.tile([128, TPC], F32, name="ff", tag="ff")
                            if fp8:
                                for jp in range(JCH // 2):
                                    nc.tensor.matmul(ff, w2t[:, jp, :, :], g_sb[jp],
                                                     start=(jp == 0),
                                                     stop=(jp == JCH // 2 - 1),
                                                     perf_mode=DR)
                            else:
                                for jc in range(JCH):
                                    nc.tensor.matmul(ff, w2t[:, jc, :], g_sb[jc],
                                                     start=(jc == 0), stop=(jc == JCH - 1))
                            xr = ffn_t.tile([128, TPC], F32, name="xr", tag="xr")
                            nc.sync.dma_start(out=xr, in_=xT[s][kc * 128:(kc + 1) * 128, :])
                            fr = ffn_h.tile([128, TPC], F32, name=f"fr{s}{kc}",
                                            tag=f"fr{s}{kc}")
                            if fp8:
                                xr2 = ffn_t.tile([128, TPC], F32, name="xr2", tag="xr2")
                                nc.scalar.add(xr2, xr, b2_sb[s][:, kc:kc + 1])
                                nc.vector.scalar_tensor_tensor(
                                    out=fr, in0=ff, scalar=1.0 / (FSC * FSC), in1=xr2,
                                    op0=AL.mult, op1=AL.add)
                            else:
                                nc.vector.scalar_tensor_tensor(
                                    out=fr, in0=ff, scalar=b2_sb[s][:, kc:kc + 1],
                                    in1=xr, op0=AL.add, op1=AL.add)
                            ffr.append(fr)
                            sq = ffn_t.tile([128, TPC], F32, name="fsq", tag="fsq")
                            nc.scalar.activation(out=sq, in_=fr, func=AF.Square)
                            nc.tensor.matmul(ms_ps, ones_f, sq,
                                             start=(kc == 0), stop=(kc == KCH - 1))
                        sd = ffn_t.tile([1, TPC], F32, name="fsd", tag="fsd")
                        nc.scalar.activation(out=sd, in_=ms_ps, func=AF.Sqrt,
                                             bias=eps_sb[0:1, :], scale=1.0 / D)
                        rec = ffn_t.tile([1, TPC], F32, name="frec", tag="frec")
                        nc.vector.reciprocal(out=rec, in_=sd)
                        rb_ps = ffn_ps1.tile([128, TPC], F32, name="frb", tag="aux")
                        nc.tensor.matmul(rb_ps, ones_row, rec, start=True, stop=True)
                        rb = ffn_t.tile([128, TPC], F32, name="frbs", tag="frbs")
                        nc.scalar.copy(out=rb, in_=rb_ps)
                        si = 0 if s == "x" else 1
                        for kc in range(KCH):
                            ot = ffn_t.tile([128, TPC], F32, name="ot", tag="ot")
                            nc.vector.scalar_tensor_tensor(
                                out=ot, in0=ffr[kc], scalar=fnw_sb[s][:, kc:kc + 1],
                                in1=rb, op0=AL.mult, op1=AL.mult)
                            nc.sync.dma_start(
                                out=out_ext[si, kc * 128:(kc + 1) * 128, :], in_=ot)

    nc.compile()
    return nc


def prepare_in_maps(inputs):
    perm = _rope_perm()
    x = np.asarray(inputs["x"], np.float32).reshape(T, D)
    y = np.asarray(inputs["y"], np.float32).reshape(T, D)
    cos = np.asarray(inputs["freqs_cos"], np.float32).T  # [64, S]
    sin = np.asarray(inputs["freqs_sin"], np.float32).T
    cs = np.concatenate([cos, cos], 0)                   # [128, S]
    sn = np.concatenate([-sin, sin], 0)
    sc = 1.0 / math.sqrt(HD)

    common = {
        "cs_q": cs * sc, "sn_q": sn * sc,
        "cs_k": cs, "sn_k": sn,
        "anw": np.asarray(inputs["attn_norm_w"], np.float32).reshape(KCH, 128),
    }

    def tile_lhs(w):  # [K, M] -> [M//128, 128(part=K%), K//128, 128] tiles
        K, M = w.shape
        return np.ascontiguousarray(
            w.reshape(K // 128, 128, M // 128, 128).transpose(2, 1, 0, 3)
        ).astype(BF)

    for s in ("x", "y"):
        if USE_FP8:
            def tile_f8(w):
                K_, M_ = w.shape
                return np.ascontiguousarray(
                    (w * FSC).reshape(K_ // 128, 128, M_ // 128, 128)
                    .transpose(2, 1, 0, 3)).astype(E4)
            common[f"w1_{s}"] = tile_f8(np.asarray(inputs[f"W1_{s}"], np.float32))
            common[f"w3_{s}"] = tile_f8(np.asarray(inputs[f"W3_{s}"], np.float32))
            common[f"w2_{s}"] = tile_f8(np.asarray(inputs[f"W2_{s}"], np.float32))
        else:
            common[f"w1_{s}"] = tile_lhs(np.asarray(inputs[f"W1_{s}"], np.float32))
            common[f"w3_{s}"] = tile_lhs(np.asarray(inputs[f"W3_{s}"], np.float32))
            common[f"w2_{s}"] = tile_lhs(np.asarray(inputs[f"W2_{s}"], np.float32))
        common[f"b1_{s}"] = np.asarray(inputs[f"b1_{s}"], np.float32).reshape(JCH, 128)
        common[f"b3_{s}"] = np.asarray(inputs[f"b3_{s}"], np.float32).reshape(JCH, 128)
        if USE_FP8:
            common[f"b3_{s}"] = common[f"b3_{s}"] * (FSC * FSC)
        common[f"b2_{s}"] = np.asarray(inputs[f"b2_{s}"], np.float32).reshape(KCH, 128)
        common[f"bo_{s}"] = np.asarray(inputs[f"bo_{s}"], np.float32).reshape(KCH, 128)
        common[f"fnw_{s}"] = np.asarray(
            inputs[f"ffn_norm_w_{s}"], np.float32).reshape(KCH, 128)

    in_maps = []
    for c in range(NC):
        m = dict(common)
        m["xT"] = np.ascontiguousarray(x[c * TPC:(c + 1) * TPC].T)
        m["yT"] = np.ascontiguousarray(y[c * TPC:(c + 1) * TPC].T)
        for s in ("x", "y"):
            Wq = np.asarray(inputs[f"Wq_{s}"], np.float32)
            Wk = np.asarray(inputs[f"Wk_{s}"], np.float32)
            Wv = np.asarray(inputs[f"Wv_{s}"], np.float32)
            Wo = np.asarray(inputs[f"Wo_{s}"], np.float32)
            bqv = np.asarray(inputs[f"bq_{s}"], np.float32)
            bkv = np.asarray(inputs[f"bk_{s}"], np.float32)
            bvv = np.asarray(inputs[f"bv_{s}"], np.float32)
            hsl = [HPC * c + h for h in range(HPC)]
            # [HPC, 128(part=K%), KCH, 128] per-head rope-permuted lhsT tiles
            def tile_q(w):
                return np.ascontiguousarray(
                    w.reshape(KCH, 128, 1, 128)
                    .transpose(2, 1, 0, 3))[0].astype(BF)
            wq_t = np.stack([tile_q(Wq[:, h * HD:(h + 1) * HD][:, perm]) for h in hsl])
            wk_t = np.stack([tile_q(Wk[:, h * HD:(h + 1) * HD][:, perm]) for h in hsl])
            m[f"wq_{s}"] = wq_t
            m[f"wk_{s}"] = wk_t
            vcols = np.concatenate([Wv[:, h * HD:(h + 1) * HD] for h in hsl], 1)
            m[f"wv_{s}"] = np.ascontiguousarray(
                vcols.reshape(KCH, 128, HPC * 128)
                .transpose(1, 0, 2)).astype(BF)
            worows = np.concatenate([Wo[h * HD:(h + 1) * HD, :] for h in hsl], 0)
            m[f"wo_{s}"] = np.ascontiguousarray(
                worows.reshape(HPC, 128, KCH, 128)).astype(BF)
            bq_p = np.stack([bqv[h * HD:(h + 1) * HD][perm] for h in hsl])
            bk_p = np.stack([bkv[h * HD:(h + 1) * HD][perm] for h in hsl])
            m[f"bq_{s}"] = bq_p
            m[f"bqs_{s}"] = np.concatenate([bq_p[:, 64:], bq_p[:, :64]], 1)
            m[f"bk_{s}"] = bk_p
            m[f"bks_{s}"] = np.concatenate([bk_p[:, 64:], bk_p[:, :64]], 1)
            m[f"bv_{s}"] = np.concatenate(
                [bvv[h * HD:(h + 1) * HD] for h in hsl])
        in_maps.append(m)
    return in_maps


def get_nc():
    if "nc" not in _CACHE:
        _CACHE["nc"] = build_nc()
    return _CACHE["nc"]


def kernel(**inputs):
    nc = get_nc()
    in_maps = prepare_in_maps(inputs)
    res = run_bass_kernel_spmd(nc, in_maps, core_ids=list(range(NC)))
    outs = []
    for si in range(2):
        full = np.concatenate([r["out"][si] for r in res.results], axis=1)  # [D, T]
        outs.append(np.ascontiguousarray(full.T).reshape(B, S, D))
    return outs[0], outs[1]


if __name__ == "__main__":
    nc = get_nc()
    print("build + compile OK")



# revision 20
# speedup vs baseline: 1.0574x; 1.0574x over previous
"""Trainium2 8-core Bass kernel for a dual cross-attention transformer block.

v3 design (vs v2 baseline):
- Activations replicated: each core gets the FULL token set in bf16,
  transposed [128, KCH, T].  No AllGather of normalized activations.
- RMSNorm folded: anw folded into Wq/Wk/Wv host-side; the per-token
  rsqrt scale `rs` is computed from each core's own f32 token shard,
  AllGathered as tiny [2,512]-f32 rows (plus a column-layout copy for
  the V path), and applied post-projection (commutes through matmul).
- Wo partial-sum ReduceScatter (16.8MB/side) replaced by an AllToAll of
  attention head outputs (2.1MB/side) + local full-D Wo per token shard.
- Attention in 512-query groups; PSUM budget exactly 8 banks
  (aux 2 + qkv 2 + scores 2 + pv 2) so QKV(y) can overlap attention(x).
- Emission order: stats, QKV(x), attn(x)+A2A(x), QKV(y), Wo(x)+h8(x),
  attn(y)+A2A(y), Wo(y)+h8(y), FFN(x), FFN(y).
- FFN unchanged from v2: fp8 DoubleRow W1/W3/W2, token-parallel.
"""

import math

import numpy as np
import ml_dtypes

import concourse.bass as bass
import concourse.tile as tile
from concourse import mybir, bacc
from concourse.bass_utils import run_bass_kernel_spmd

B, S, D, H = 2, 2048, 2048, 16
HD = D // H            # 128
HID = 5632
EPS = 1e-5
NC = 8                 # cores
HPC = H // NC          # 2 heads per core
FC = D // 128          # 16 feature chunks
T = B * S              # 4096 tokens
TPC = T // NC          # 512 tokens per core
KCH = D // 128         # 16 contraction chunks over D
JCH = HID // 128       # 44 chunks over HID
QBLK = 256             # token block for QKV projections
NQB = T // QBLK        # 16
NA = TPC // 128        # 4 column-chunks per core shard
BF = ml_dtypes.bfloat16
E4 = ml_dtypes.float8_e4m3
F32 = mybir.dt.float32
BF16 = mybir.dt.bfloat16
FP8 = mybir.dt.float8e4
FSC = 16.0

_CACHE = {}


def _rope_perm():
    # [evens, odds]: puts x1 in partitions 0:64, x2 in 64:128 of Q^T/K^T
    return np.concatenate([np.arange(0, 128, 2), np.arange(1, 128, 2)])


def build_nc(sim_local=False, reps=1):
    nc = bacc.Bacc("TRN2", target_bir_lowering=False, debug=False,
                   num_devices=1 if sim_local else NC)

    AL = mybir.AluOpType
    AF = mybir.ActivationFunctionType

    # ---------------- external parameters ----------------
    xT = {s: nc.declare_dram_parameter(f"{s}T", [D, TPC], F32, isOutput=False)
          for s in ("x", "y")}
    xTf = {s: nc.declare_dram_parameter(f"{s}Tf", [128, KCH, T], BF16,
                                        isOutput=False) for s in ("x", "y")}
    tabs = {n: nc.declare_dram_parameter(n, [128, S], BF16, isOutput=False)
            for n in ("cs_q", "sn_q", "cs_k", "sn_k")}
    wq, wk, wv, wo = {}, {}, {}, {}
    bq, bqs, bk, bks, bv = {}, {}, {}, {}, {}
    w1, w3, w2, b1, b3, b2, bo, fnw = {}, {}, {}, {}, {}, {}, {}, {}
    for s in ("x", "y"):
        wq[s] = nc.declare_dram_parameter(f"wq_{s}", [HPC, 128, KCH, 128], BF16, isOutput=False)
        wk[s] = nc.declare_dram_parameter(f"wk_{s}", [HPC, 128, KCH, 128], BF16, isOutput=False)
        wv[s] = nc.declare_dram_parameter(f"wv_{s}", [128, KCH, HPC * 128], BF16, isOutput=False)
        wo[s] = nc.declare_dram_parameter(f"wo_{s}", [KCH, 128, FC, 128], BF16, isOutput=False)
        bq[s] = nc.declare_dram_parameter(f"bq_{s}", [HPC, 128], F32, isOutput=False)
        bqs[s] = nc.declare_dram_parameter(f"bqs_{s}", [HPC, 128], F32, isOutput=False)
        bk[s] = nc.declare_dram_parameter(f"bk_{s}", [HPC, 128], F32, isOutput=False)
        bks[s] = nc.declare_dram_parameter(f"bks_{s}", [HPC, 128], F32, isOutput=False)
        bv[s] = nc.declare_dram_parameter(f"bv_{s}", [HPC * 128], F32, isOutput=False)
        bo[s] = nc.declare_dram_parameter(f"bo_{s}", [KCH, 128], F32, isOutput=False)
        w1[s] = nc.declare_dram_parameter(f"w1_{s}", [JCH, 128, KCH, 128], FP8, isOutput=False)
        w3[s] = nc.declare_dram_parameter(f"w3_{s}", [JCH, 128, KCH, 128], FP8, isOutput=False)
        w2[s] = nc.declare_dram_parameter(f"w2_{s}", [KCH, 128, JCH, 128], FP8, isOutput=False)
        b1[s] = nc.declare_dram_parameter(f"b1_{s}", [JCH, 128], F32, isOutput=False)
        b3[s] = nc.declare_dram_parameter(f"b3_{s}", [JCH, 128], F32, isOutput=False)
        b2[s] = nc.declare_dram_parameter(f"b2_{s}", [KCH, 128], F32, isOutput=False)
        fnw[s] = nc.declare_dram_parameter(f"fnw_{s}", [KCH, 128], F32, isOutput=False)
    out_ext = nc.declare_dram_parameter("out", [2, D, TPC], F32, isOutput=True)

    # ---------------- internal DRAM ----------------
    rs_in = nc.dram_tensor("rs_in", [2, TPC], F32)
    rs_out = nc.dram_tensor("rs_out", [2 * NC, TPC], F32, addr_space="Shared")
    a2a_in = {s: nc.dram_tensor(f"a2a_in_{s}", [NC, HPC, 128, TPC], BF16)
              for s in ("x", "y")}
    a2a_out = {s: nc.dram_tensor(f"a2a_out_{s}", [NC, HPC, 128, TPC], BF16)
               for s in ("x", "y")}

    rg = [list(range(NC))]

    from contextlib import ExitStack
    with tile.TileContext(nc) as tc:
        with ExitStack() as es:
            const = es.enter_context(tc.tile_pool(name="const", bufs=1, side="left"))
            ones_bf = const.tile([128, 1], BF16)
            nc.vector.memset(ones_bf, 1.0)
            ones_f = const.tile([128, 1], F32)
            nc.vector.memset(ones_f, 1.0)
            ones_row = const.tile([1, 128], F32)
            nc.vector.memset(ones_row, 1.0)
            eps_sb = const.tile([128, 1], F32)
            nc.vector.memset(eps_sb, EPS)
            fnw_sb, bo_sb, b2_sb = {}, {}, {}
            bq_sb, bqs_sb, bk_sb, bks_sb, bv_sb = {}, {}, {}, {}, {}
            b1_sb, b3_sb = {}, {}
            for s in ("x", "y"):
                fnw_sb[s] = const.tile([128, KCH], F32, name=f"fnw{s}", tag=f"fnw{s}")
                nc.sync.dma_start(out=fnw_sb[s], in_=fnw[s].rearrange("k p -> p k"))
                bo_sb[s] = const.tile([128, KCH], F32, name=f"bo{s}", tag=f"bo{s}")
                nc.sync.dma_start(out=bo_sb[s], in_=bo[s].rearrange("k p -> p k"))
                b2_sb[s] = const.tile([128, KCH], F32, name=f"b2{s}", tag=f"b2{s}")
                nc.sync.dma_start(out=b2_sb[s], in_=b2[s].rearrange("k p -> p k"))
                b1_sb[s] = const.tile([128, JCH], F32, name=f"b1{s}", tag=f"b1{s}")
                nc.sync.dma_start(out=b1_sb[s], in_=b1[s].rearrange("k p -> p k"))
                b3_sb[s] = const.tile([128, JCH], F32, name=f"b3{s}", tag=f"b3{s}")
                nc.sync.dma_start(out=b3_sb[s], in_=b3[s].rearrange("k p -> p k"))
                bq_sb[s] = const.tile([128, HPC], F32, name=f"bq{s}", tag=f"bq{s}")
                nc.sync.dma_start(out=bq_sb[s], in_=bq[s].rearrange("h p -> p h"))
                bqs_sb[s] = const.tile([128, HPC], F32, name=f"bqs{s}", tag=f"bqs{s}")
                nc.sync.dma_start(out=bqs_sb[s], in_=bqs[s].rearrange("h p -> p h"))
                bk_sb[s] = const.tile([128, HPC], F32, name=f"bk{s}", tag=f"bk{s}")
                nc.sync.dma_start(out=bk_sb[s], in_=bk[s].rearrange("h p -> p h"))
                bks_sb[s] = const.tile([128, HPC], F32, name=f"bks{s}", tag=f"bks{s}")
                nc.sync.dma_start(out=bks_sb[s], in_=bks[s].rearrange("h p -> p h"))
                bv_sb[s] = const.tile([128, HPC * 128], F32, name=f"bv{s}", tag=f"bv{s}")
                nc.sync.dma_start(out=bv_sb[s],
                                  in_=bv[s][None, :].to_broadcast([128, HPC * 128]))

            for _rep in range(reps):
                aux_es = ExitStack()
                aux_ps = aux_es.enter_context(
                    tc.tile_pool(name="aux_ps", bufs=2, space="PSUM", side="left"))

                # ---------- phase 0: own-shard stats + tiny AllGathers ----------
                with tc.tile_pool(name="nrm", bufs=3, side="left") as nrm:
                    for si, s in enumerate(("x", "y")):
                        ms_ps = aux_ps.tile([1, TPC], F32, name="ms", tag="aux")
                        for kc in range(KCH):
                            t = nrm.tile([128, TPC], F32, name="xt", tag="xt")
                            nc.sync.dma_start(out=t, in_=xT[s][kc * 128:(kc + 1) * 128, :])
                            sq = nrm.tile([128, TPC], F32, name="sq", tag="sq")
                            nc.scalar.activation(out=sq, in_=t, func=AF.Square)
                            nc.tensor.matmul(ms_ps, ones_f, sq,
                                             start=(kc == 0), stop=(kc == KCH - 1))
                        sd = nrm.tile([1, TPC], F32, name="sd", tag="sd")
                        nc.scalar.activation(out=sd, in_=ms_ps, func=AF.Sqrt,
                                             bias=eps_sb[0:1, :], scale=1.0 / D)
                        rec = nrm.tile([1, TPC], F32, name="rec", tag="rec", bufs=2)
                        nc.vector.reciprocal(out=rec, in_=sd)
                        nc.sync.dma_start(out=rs_in[si:si + 1, :], in_=rec)
                if sim_local:
                    for g in range(NC):
                        nc.sync.dma_start(out=rs_out[2 * g:2 * g + 2, :], in_=rs_in[:])
                else:
                    nc.gpsimd.collective_compute(
                        "AllGather", AL.bypass, replica_groups=rg,
                        ins=[rs_in[:]], outs=[rs_out[:]])
                # column view rsc_sb[p, (c s a)] = rs_out[2c+s, a*128+p]
                rsc_sb = const.tile([128, 2 * NC * NA], F32, name="rsc", tag="rsc")
                nc.sync.dma_start(
                    out=rsc_sb,
                    in_=rs_out.rearrange("(c s) (a p) -> p (c s a)", s=2, p=128))

                # FFN-input tiles (outlive attention/Wo phases); right stack
                ffn_h = aux_es.enter_context(
                    tc.tile_pool(name="ffn_h", bufs=1, side="right"))
                h8 = {s: [ffn_h.tile([128, 2, TPC], FP8, name=f"h8{s}_{i}",
                                     tag=f"h8{s}_{i}") for i in range(KCH // 2)]
                      for s in ("x", "y")}
                ffn_h.seal()

                # ---------- persistent per-side QKV output tiles ----------
                qkt_cm, vp_cm = {}, {}
                qkt, vp = {}, {}
                qt_sb, kt_sb, v_sb = {}, {}, {}
                for s in ("y", "x"):
                    qkt_cm[s] = tc.tile_pool(name=f"qkt{s}", bufs=1, side="right")
                    qkt[s] = qkt_cm[s].__enter__()
                    vp_cm[s] = tc.tile_pool(name=f"vp{s}", bufs=1, side="right")
                    vp[s] = vp_cm[s].__enter__()
                    for h in range(HPC):
                        qt_sb[(s, h)] = qkt[s].tile([128, T], BF16,
                                                    name=f"qt{s}{h}", tag=f"qt{s}{h}")
                        kt_sb[(s, h)] = qkt[s].tile([128, T], BF16,
                                                    name=f"kt{s}{h}", tag=f"kt{s}{h}")
                    v_sb[s] = vp[s].tile([128, T // 128, HPC * 128], BF16,
                                         name=f"v{s}", tag=f"v{s}")

                qkv_es = ExitStack()
                qkvw = qkv_es.enter_context(tc.tile_pool(name="qkvw", bufs=1, side="right"))
                wq_sb, wk_sb, wv_sb = {}, {}, {}
                for s in ("x", "y"):
                    for h in range(HPC):
                        wq_sb[(s, h)] = qkvw.tile([128, KCH, 128], BF16, name=f"wq{s}{h}", tag=f"wq{s}{h}")
                        nc.sync.dma_start(out=wq_sb[(s, h)], in_=wq[s][h])
                        wk_sb[(s, h)] = qkvw.tile([128, KCH, 128], BF16, name=f"wk{s}{h}", tag=f"wk{s}{h}")
                        nc.sync.dma_start(out=wk_sb[(s, h)], in_=wk[s][h])
                    wv_sb[s] = qkvw.tile([128, KCH, HPC * 128], BF16, name=f"wv{s}", tag=f"wv{s}")
                    nc.sync.dma_start(out=wv_sb[s], in_=wv[s][:])

                acts = qkv_es.enter_context(tc.tile_pool(name="acts", bufs=2, side="right"))
                ropet = qkv_es.enter_context(tc.tile_pool(name="ropet", bufs=2, side="right"))
                ropes = qkv_es.enter_context(tc.tile_pool(name="ropes", bufs=2, side="right"))
                qkv_ps = qkv_es.enter_context(
                    tc.tile_pool(name="qkv_ps", bufs=2, space="PSUM", side="right"))

                def emit_qkv_side(s):
                    so = "y" if s == "x" else "x"
                    si_s = 0 if s == "x" else 1
                    si_o = 1 - si_s
                    for qb in range(NQB):
                        pos = (qb * QBLK) % S
                        a_q = acts.tile([128, KCH, QBLK], BF16, name="aq", tag="a")
                        nc.sync.dma_start(
                            out=a_q, in_=xTf[so][:, :, qb * QBLK:(qb + 1) * QBLK])
                        a_kv = acts.tile([128, KCH, QBLK], BF16, name="akv", tag="a")
                        nc.sync.dma_start(
                            out=a_kv, in_=xTf[s][:, :, qb * QBLK:(qb + 1) * QBLK])
                        tb = {}
                        for n in ("cs_q", "sn_q", "cs_k", "sn_k"):
                            tt = ropet.tile([128, QBLK], BF16, tag=n)
                            nc.sync.dma_start(out=tt, in_=tabs[n][:, pos:pos + QBLK])
                            tb[n] = tt
                        rb = {}
                        for srci in (si_s, si_o):
                            row = 2 * (qb // 2) + srci
                            col = (qb % 2) * QBLK
                            rbt = ropes.tile([128, QBLK], F32, name=f"rb{srci}",
                                             tag=f"rb{srci}", bufs=1)
                            nc.sync.dma_start(
                                out=rbt,
                                in_=rs_out[row:row + 1,
                                           col:col + QBLK].to_broadcast(
                                               [128, QBLK]))
                            rb[srci] = rbt
                        for h in range(HPC):
                            for proj, wsb, bsb, bssb, cs_t, sn_t, dst, rbt in (
                                ("q", wq_sb[(s, h)], bq_sb[s], bqs_sb[s],
                                 tb["cs_q"], tb["sn_q"], qt_sb[(s, h)], rb[si_o]),
                                ("k", wk_sb[(s, h)], bk_sb[s], bks_sb[s],
                                 tb["cs_k"], tb["sn_k"], kt_sb[(s, h)], rb[si_s]),
                            ):
                                src_a = a_q if proj == "q" else a_kv
                                ps = qkv_ps.tile([128, QBLK], F32, name="qk", tag="qv")
                                for kc in range(KCH):
                                    nc.tensor.matmul(
                                        ps, wsb[:, kc, :], src_a[:, kc, :],
                                        start=(kc == 0), stop=(kc == KCH - 1))
                                # dst = (ps*rs + b)*cs + (swap(ps*rs) + bs)*sn
                                qs = ropes.tile([128, QBLK], F32, name="qs", tag="qs")
                                nc.vector.tensor_mul(qs, ps, rbt)
                                qsw = ropes.tile([128, QBLK], F32, name="qsw", tag="qsw")
                                nc.sync.dma_start(out=qsw[0:64, :], in_=qs[64:128, :])
                                nc.sync.dma_start(out=qsw[64:128, :], in_=qs[0:64, :])
                                t1 = ropes.tile([128, QBLK], F32, name="t1", tag="t1")
                                nc.vector.scalar_tensor_tensor(
                                    out=t1, in0=qs, scalar=bsb[:, h:h + 1],
                                    in1=cs_t, op0=AL.add, op1=AL.mult)
                                t2 = ropes.tile([128, QBLK], F32, name="t2", tag="t2")
                                nc.vector.scalar_tensor_tensor(
                                    out=t2, in0=qsw, scalar=bssb[:, h:h + 1],
                                    in1=sn_t, op0=AL.add, op1=AL.mult)
                                nc.vector.tensor_add(
                                    dst[:, qb * QBLK:(qb + 1) * QBLK], t1, t2)
                        # V in natural [token, hd] layout; rs per-partition here
                        for tk in range(QBLK // 128):
                            ci = ((qb // 2) * 2 + si_s) * NA + 2 * (qb % 2) + tk
                            vps = qkv_ps.tile([128, HPC * 128], F32, name="v", tag="qv")
                            for kc in range(KCH):
                                nc.tensor.matmul(
                                    vps, a_kv[:, kc, tk * 128:(tk + 1) * 128],
                                    wv_sb[s][:, kc, :],
                                    start=(kc == 0), stop=(kc == KCH - 1))
                            nc.vector.scalar_tensor_tensor(
                                out=v_sb[s][:, qb * 2 + tk, :], in0=vps,
                                scalar=rsc_sb[:, ci:ci + 1],
                                in1=bv_sb[s], op0=AL.mult, op1=AL.add)

                def emit_attn_side(s, att_pt, att_sb, att_ps, att_po):
                    for b in range(B):
                        for gq in range(S // 512):
                            tq0 = b * S + gq * 512
                            g = tq0 // 512
                            for h in range(HPC):
                                po = att_po.tile([128, 512], F32, name="po", tag="po")
                                stack = []
                                for tkc in range(16):
                                    sps = att_ps.tile([128, 512], F32, name="s", tag="s")
                                    nc.tensor.matmul(
                                        sps,
                                        kt_sb[(s, h)][:, b * S + tkc * 128:
                                                      b * S + (tkc + 1) * 128],
                                        qt_sb[(s, h)][:, tq0:tq0 + 512],
                                        start=True, stop=True)
                                    p = att_pt.tile([128, 512], BF16, name="pt",
                                                    tag="pt")
                                    nc.scalar.activation(out=p, in_=sps, func=AF.Exp)
                                    nc.tensor.matmul(
                                        po,
                                        v_sb[s][:, b * (S // 128) + tkc,
                                                h * 128:(h + 1) * 128],
                                        p, start=(tkc == 0), stop=(tkc == 15))
                                    cur = (0, p)
                                    while stack and stack[-1][0] == cur[0]:
                                        lvl, other = stack.pop()
                                        dsum = att_pt.tile([128, 512], BF16,
                                                           name="ds", tag="pt")
                                        nc.vector.tensor_add(dsum, other, cur[1])
                                        cur = (lvl + 1, dsum)
                                    stack.append(cur)
                                lvl_final = stack[-1][1]
                                dn = aux_ps.tile([1, 512], F32, name="dn", tag="aux")
                                nc.tensor.matmul(dn, ones_bf, lvl_final,
                                                 start=True, stop=True)
                                rec = att_sb.tile([1, 512], F32, name="rec", tag="rec", bufs=1)
                                nc.vector.reciprocal(out=rec, in_=dn)
                                rbb = aux_ps.tile([128, 512], F32, name="rbb", tag="aux")
                                nc.tensor.matmul(rbb, ones_row, rec,
                                                 start=True, stop=True)
                                rbs = att_sb.tile([128, 512], BF16, name="rbs",
                                                  tag="rbs", bufs=1)
                                nc.vector.tensor_copy(out=rbs, in_=rbb)
                                ot = att_sb.tile([128, 512], BF16, name="ot",
                                                 tag="ot", bufs=3)
                                nc.vector.tensor_mul(ot, po, rbs)
                                nc.sync.dma_start(out=a2a_in[s][g, h], in_=ot)
                    if sim_local:
                        nc.sync.dma_start(out=a2a_out[s][:], in_=a2a_in[s][:])
                    else:
                        nc.gpsimd.collective_compute(
                            "AllToAll", AL.bypass, replica_groups=rg,
                            ins=[a2a_in[s][:]], outs=[a2a_out[s][:]])

                def emit_wo_side(s, wo_w, wo_ps, ffn_h, h8):
                    o_sb = wo_w.tile([128, FC, TPC], BF16, name=f"o{s}", tag=f"o{s}")
                    nc.sync.dma_start(out=o_sb,
                                      in_=a2a_out[s].rearrange("c h p t -> p (c h) t"))
                    for kc in range(KCH):
                        wot = wo_w.tile([128, FC, 128], BF16, name="wot",
                                        tag="wot", bufs=2)
                        nc.sync.dma_start(out=wot, in_=wo[s][kc])
                        wps = wo_ps.tile([128, TPC], F32, name="wps", tag="wps")
                        for fc in range(FC):
                            nc.tensor.matmul(wps, wot[:, fc, :], o_sb[:, fc, :],
                                             start=(fc == 0), stop=(fc == FC - 1))
                        nc.vector.tensor_scalar(
                            h8[s][kc // 2][:, kc % 2, :], wps,
                            bo_sb[s][:, kc:kc + 1], FSC,
                            op0=AL.add, op1=AL.mult)

                # ---- emission ----
                emit_qkv_side("x")

                att_es = ExitStack()
                att_pt = att_es.enter_context(tc.tile_pool(name="att_pt", bufs=7, side="left"))
                att_sb = att_es.enter_context(tc.tile_pool(name="att_sb", bufs=2, side="left"))
                att_ps = att_es.enter_context(
                    tc.tile_pool(name="att_ps", bufs=2, space="PSUM", side="left"))
                att_po = att_es.enter_context(
                    tc.tile_pool(name="att_po", bufs=2, space="PSUM", side="left"))

                emit_attn_side("x", att_pt, att_sb, att_ps, att_po)
                emit_qkv_side("y")
                qkv_es.close()
                vp_cm["x"].__exit__(None, None, None)
                qkt_cm["x"].__exit__(None, None, None)

                wo_es = ExitStack()
                wo_w = wo_es.enter_context(tc.tile_pool(name="wo_w", bufs=1, side="right"))
                wo_ps = wo_es.enter_context(
                    tc.tile_pool(name="wo_ps", bufs=2, space="PSUM", side="right"))
                emit_wo_side("x", wo_w, wo_ps, ffn_h, h8)
                emit_attn_side("y", att_pt, att_sb, att_ps, att_po)
                emit_wo_side("y", wo_w, wo_ps, ffn_h, h8)
                att_es.close()
                wo_es.close()
                vp_cm["y"].__exit__(None, None, None)
                qkt_cm["y"].__exit__(None, None, None)

                # ---------- FFN + residual + final norm per side ----------
                with tc.tile_pool(name="ffn_g", bufs=1, side="left") as ffn_g, \
                     tc.tile_pool(name="ffn_w", bufs=3, side="left") as ffn_w, \
                     tc.tile_pool(name="ffn_t", bufs=2, side="left") as ffn_t, \
                     tc.tile_pool(name="ffn_fr", bufs=1, side="left") as ffn_fr, \
                     tc.tile_pool(name="ffn_ps", bufs=2, space="PSUM", side="left") as ffn_ps:
                    DR = mybir.MatmulPerfMode.DoubleRow
                    for s in ("x", "y"):
                        g_sb = []
                        for jc in range(JCH):
                            w1t = ffn_w.tile([128, KCH // 2, 2, 128], FP8,
                                             name="w1", tag="w1")
                            nc.sync.dma_start(out=w1t, in_=w1[s][jc])
                            w3t = ffn_w.tile([128, KCH // 2, 2, 128], FP8,
                                             name="w3", tag="w3")
                            nc.sync.dma_start(out=w3t, in_=w3[s][jc])
                            z1 = ffn_ps.tile([128, TPC], F32, name="z1", tag="z", bufs=3)
                            z3 = ffn_ps.tile([128, TPC], F32, name="z3", tag="z", bufs=3)
                            for kp in range(KCH // 2):
                                nc.tensor.matmul(z1, w1t[:, kp, :, :], h8[s][kp],
                                                 start=(kp == 0),
                                                 stop=(kp == KCH // 2 - 1),
                                                 perf_mode=DR)
                            for kp in range(KCH // 2):
                                nc.tensor.matmul(z3, w3t[:, kp, :, :], h8[s][kp],
                                                 start=(kp == 0),
                                                 stop=(kp == KCH // 2 - 1),
                                                 perf_mode=DR)
                            sz = ffn_t.tile([128, TPC], F32, name="sz", tag="sz")
                            nc.scalar.activation(out=sz, in_=z1, func=AF.Silu,
                                                 bias=b1_sb[s][:, jc:jc + 1],
                                                 scale=1.0 / (FSC * FSC))
                            gt = ffn_g.tile([128, TPC], BF16, name="gt",
                                            tag="gt", bufs=2)
                            nc.vector.scalar_tensor_tensor(
                                out=gt, in0=z3, scalar=b3_sb[s][:, jc:jc + 1], in1=sz,
                                op0=AL.add, op1=AL.mult)
                            if jc % 2 == 0:
                                g8 = ffn_g.tile([128, 2, TPC], FP8,
                                                name=f"g8{s}_{jc // 2}",
                                                tag=f"g8{s}_{jc // 2}")
                                g_sb.append(g8)
                            nc.scalar.mul(out=g_sb[jc // 2][:, jc % 2, :],
                                          in_=gt, mul=1.0 / FSC)
                        # W2 pass + residual + stats
                        ffr = []
                        ms_ps = aux_ps.tile([1, TPC], F32, name="ms2", tag="aux")
                        for kc in range(KCH):
                            w2t = ffn_w.tile([128, JCH // 2, 2, 128], FP8,
                                             name="w2", tag="w2", bufs=2)
                            nc.sync.dma_start(out=w2t, in_=w2[s][kc])
                            ff = ffn_ps.tile([128, TPC], F32, name="ff", tag="ff")
                            for jp in range(JCH // 2):
                                nc.tensor.matmul(ff, w2t[:, jp, :, :], g_sb[jp],
                                                 start=(jp == 0),
                                                 stop=(jp == JCH // 2 - 1),
                                                 perf_mode=DR)
                            xr = ffn_t.tile([128, TPC], F32, name="xr", tag="xr")
                            nc.sync.dma_start(out=xr,
                                              in_=xT[s][kc * 128:(kc + 1) * 128, :])
                            fr = ffn_fr.tile([128, TPC], F32, name=f"fr{s}{kc}",
                                             tag=f"fr{s}{kc}")
                            xr2 = ffn_t.tile([128, TPC], F32, name="xr2", tag="xr2")
                            nc.scalar.add(xr2, xr, b2_sb[s][:, kc:kc + 1])
                            nc.vector.scalar_tensor_tensor(
                                out=fr, in0=ff, scalar=1.0 / (FSC * FSC), in1=xr2,
                                op0=AL.mult, op1=AL.add)
                            ffr.append(fr)
                            sq = ffn_t.tile([128, TPC], F32, name="fsq", tag="fsq")
                            nc.scalar.activation(out=sq, in_=fr, func=AF.Square)
                            nc.tensor.matmul(ms_ps, ones_f, sq,
                                             start=(kc == 0), stop=(kc == KCH - 1))
                        sd = ffn_t.tile([1, TPC], F32, name="fsd", tag="fsd")
                        nc.scalar.activation(out=sd, in_=ms_ps, func=AF.Sqrt,
                                             bias=eps_sb[0:1, :], scale=1.0 / D)
                        rec = ffn_t.tile([1, TPC], F32, name="frec", tag="frec")
                        nc.vector.reciprocal(out=rec, in_=sd)
                        rb_ps = aux_ps.tile([128, TPC], F32, name="frb", tag="aux")
                        nc.tensor.matmul(rb_ps, ones_row, rec, start=True, stop=True)
                        rb = ffn_t.tile([128, TPC], F32, name="frbs", tag="frbs")
                        nc.scalar.copy(out=rb, in_=rb_ps)
                        si = 0 if s == "x" else 1
                        for kc in range(KCH):
                            ot = ffn_t.tile([128, TPC], F32, name="oto", tag="oto")
                            nc.vector.scalar_tensor_tensor(
                                out=ot, in0=ffr[kc], scalar=fnw_sb[s][:, kc:kc + 1],
                                in1=rb, op0=AL.mult, op1=AL.mult)
                            nc.sync.dma_start(
                                out=out_ext[si, kc * 128:(kc + 1) * 128, :], in_=ot)
                aux_es.close()

    nc.compile()
    return nc


def prepare_in_maps(inputs):
    perm = _rope_perm()
    x = np.asarray(inputs["x"], np.float32).reshape(T, D)
    y = np.asarray(inputs["y"], np.float32).reshape(T, D)
    anw = np.asarray(inputs["attn_norm_w"], np.float32)
    cos = np.asarray(inputs["freqs_cos"], np.float32).T  # [64, S]
    sin = np.asarray(inputs["freqs_sin"], np.float32).T
    cs = np.concatenate([cos, cos], 0)                   # [128, S]
    sn = np.concatenate([-sin, sin], 0)
    sc = 1.0 / math.sqrt(HD)

    common = {
        "cs_q": (cs * sc).astype(BF), "sn_q": (sn * sc).astype(BF),
        "cs_k": cs.astype(BF), "sn_k": sn.astype(BF),
    }
    # replicated raw activations, feature-major bf16 [128, KCH, T]
    for name, arr in (("xTf", x), ("yTf", y)):
        common[name] = np.ascontiguousarray(
            arr.T.reshape(KCH, 128, T).transpose(1, 0, 2)).astype(BF)

    def tile_f8(w):
        K_, M_ = w.shape
        return np.ascontiguousarray(
            (w * FSC).reshape(K_ // 128, 128, M_ // 128, 128)
            .transpose(2, 1, 0, 3)).astype(E4)

    percore = [dict() for _ in range(NC)]
    for s in ("x", "y"):
        common[f"w1_{s}"] = tile_f8(np.asarray(inputs[f"W1_{s}"], np.float32))
        common[f"w3_{s}"] = tile_f8(np.asarray(inputs[f"W3_{s}"], np.float32))
        common[f"w2_{s}"] = tile_f8(np.asarray(inputs[f"W2_{s}"], np.float32))
        common[f"b1_{s}"] = np.asarray(inputs[f"b1_{s}"], np.float32).reshape(JCH, 128)
        common[f"b3_{s}"] = (np.asarray(inputs[f"b3_{s}"], np.float32)
                             .reshape(JCH, 128) * (FSC * FSC))
        common[f"b2_{s}"] = np.asarray(inputs[f"b2_{s}"], np.float32).reshape(KCH, 128)
        common[f"bo_{s}"] = np.asarray(inputs[f"bo_{s}"], np.float32).reshape(KCH, 128)
        common[f"fnw_{s}"] = np.asarray(
            inputs[f"ffn_norm_w_{s}"], np.float32).reshape(KCH, 128)
        Wo = np.asarray(inputs[f"Wo_{s}"], np.float32)
        common[f"wo_{s}"] = np.ascontiguousarray(
            Wo.reshape(FC, 128, KCH, 128).transpose(2, 1, 0, 3)).astype(BF)
        Wq = np.asarray(inputs[f"Wq_{s}"], np.float32) * anw[:, None]
        Wk = np.asarray(inputs[f"Wk_{s}"], np.float32) * anw[:, None]
        Wv = np.asarray(inputs[f"Wv_{s}"], np.float32) * anw[:, None]
        bqv = np.asarray(inputs[f"bq_{s}"], np.float32)
        bkv = np.asarray(inputs[f"bk_{s}"], np.float32)
        bvv = np.asarray(inputs[f"bv_{s}"], np.float32)

        def tile_q(w):
            return np.ascontiguousarray(
                w.reshape(KCH, 128, 1, 128).transpose(2, 1, 0, 3))[0].astype(BF)

        for c in range(NC):
            hsl = [HPC * c + h for h in range(HPC)]
            m = percore[c]
            m[f"wq_{s}"] = np.stack(
                [tile_q(Wq[:, h * HD:(h + 1) * HD][:, perm]) for h in hsl])
            m[f"wk_{s}"] = np.stack(
                [tile_q(Wk[:, h * HD:(h + 1) * HD][:, perm]) for h in hsl])
            vcols = np.concatenate([Wv[:, h * HD:(h + 1) * HD] for h in hsl], 1)
            m[f"wv_{s}"] = np.ascontiguousarray(
                vcols.reshape(KCH, 128, HPC * 128).transpose(1, 0, 2)).astype(BF)
            bq_p = np.stack([bqv[h * HD:(h + 1) * HD][perm] for h in hsl])
            bk_p = np.stack([bkv[h * HD:(h + 1) * HD][perm] for h in hsl])
            m[f"bq_{s}"] = bq_p
            m[f"bqs_{s}"] = np.concatenate([bq_p[:, 64:], bq_p[:, :64]], 1)
            m[f"bk_{s}"] = bk_p
            m[f"bks_{s}"] = np.concatenate([bk_p[:, 64:], bk_p[:, :64]], 1)
            m[f"bv_{s}"] = np.concatenate([bvv[h * HD:(h + 1) * HD] for h in hsl])

    in_maps = []
    for c in range(NC):
        m = dict(common)
        m.update(percore[c])
        m["xT"] = np.ascontiguousarray(x[c * TPC:(c + 1) * TPC].T)
        m["yT"] = np.ascontiguousarray(y[c * TPC:(c + 1) * TPC].T)
        in_maps.append(m)
    return in_maps


def get_nc():
    if "nc" not in _CACHE:
        _CACHE["nc"] = build_nc()
    return _CACHE["nc"]


def kernel(**inputs):
    nc = get_nc()
    in_maps = prepare_in_maps(inputs)
    res = run_bass_kernel_spmd(nc, in_maps, core_ids=list(range(NC)))
    outs = []
    for si in range(2):
        full = np.concatenate([r["out"][si] for r in res.results], axis=1)  # [D, T]
        outs.append(np.ascontiguousarray(full.T).reshape(B, S, D))
    return outs[0], outs[1]


if __name__ == "__main__":
    nc = get_nc()
    print("build + compile OK")


# revision 24
# speedup vs baseline: 1.3197x; 1.2481x over previous
"""Trainium2 8-core Bass kernel for a dual cross-attention transformer block.

v3 design (vs v2 baseline):
- Activations replicated: each core gets the FULL token set in bf16,
  transposed [128, KCH, T].  No AllGather of normalized activations.
- RMSNorm folded: anw folded into Wq/Wk/Wv host-side; the per-token
  rsqrt scale `rs` is computed from each core's own f32 token shard,
  AllGathered as tiny [2,512]-f32 rows (plus a column-layout copy for
  the V path), and applied post-projection (commutes through matmul).
- Wo partial-sum ReduceScatter (16.8MB/side) replaced by an AllToAll of
  attention head outputs (2.1MB/side) + local full-D Wo per token shard.
- Attention in 512-query groups; PSUM budget exactly 8 banks
  (aux 2 + qkv 2 + scores 2 + pv 2) so QKV(y) can overlap attention(x).
- Emission order: stats, QKV(x), attn(x)+A2A(x), QKV(y), Wo(x)+h8(x),
  attn(y)+A2A(y), Wo(y)+h8(y), FFN(x), FFN(y).
- FFN unchanged from v2: fp8 DoubleRow W1/W3/W2, token-parallel.
"""

import math

import numpy as np
import ml_dtypes

import concourse.bass as bass
import concourse.tile as tile
from concourse import mybir, bacc
from concourse.bass_utils import run_bass_kernel_spmd

B, S, D, H = 2, 2048, 2048, 16
HD = D // H            # 128
HID = 5632
EPS = 1e-5
NC = 8                 # cores
HPC = H // NC          # 2 heads per core
FC = D // 128          # 16 feature chunks
T = B * S              # 4096 tokens
TPC = T // NC          # 512 tokens per core
KCH = D // 128         # 16 contraction chunks over D
JCH = HID // 128       # 44 chunks over HID
QBLK = 256             # token block for QKV projections
NQB = T // QBLK        # 16
NA = TPC // 128        # 4 column-chunks per core shard
BF = ml_dtypes.bfloat16
E4 = ml_dtypes.float8_e4m3
F32 = mybir.dt.float32
BF16 = mybir.dt.bfloat16
FP8 = mybir.dt.float8e4
FSC = 16.0

_CACHE = {}


def _rope_perm():
    # [evens, odds]: puts x1 in partitions 0:64, x2 in 64:128 of Q^T/K^T
    return np.concatenate([np.arange(0, 128, 2), np.arange(1, 128, 2)])


def build_nc(sim_local=False, reps=1):
    nc = bacc.Bacc("TRN2", target_bir_lowering=False, debug=False,
                   num_devices=1 if sim_local else NC)

    AL = mybir.AluOpType
    AF = mybir.ActivationFunctionType

    # ---------------- external parameters ----------------
    xT = {s: nc.declare_dram_parameter(f"{s}T", [D, TPC], F32, isOutput=False)
          for s in ("x", "y")}
    xTf = {s: nc.declare_dram_parameter(f"{s}Tf", [128, KCH // 2, 2, T], FP8,
                                        isOutput=False) for s in ("x", "y")}
    tabs = {n: nc.declare_dram_parameter(n, [128, S], BF16, isOutput=False)
            for n in ("cs_q", "sn_q", "cs_k", "sn_k")}
    wq, wk, wv, wo = {}, {}, {}, {}
    bq, bqs, bk, bks, bv = {}, {}, {}, {}, {}
    w1, w3, w2, b1, b3, b2, bo, fnw = {}, {}, {}, {}, {}, {}, {}, {}
    for s in ("x", "y"):
        wq[s] = nc.declare_dram_parameter(f"wq_{s}", [HPC, 128, KCH // 2, 2, 128], FP8, isOutput=False)
        wk[s] = nc.declare_dram_parameter(f"wk_{s}", [HPC, 128, KCH // 2, 2, 128], FP8, isOutput=False)
        wv[s] = nc.declare_dram_parameter(f"wv_{s}", [128, KCH // 2, 2, HPC * 128], FP8, isOutput=False)
        wo[s] = nc.declare_dram_parameter(f"wo_{s}", [KCH, 128, FC // 2, 2, 128], FP8, isOutput=False)
        bq[s] = nc.declare_dram_parameter(f"bq_{s}", [HPC, 128], F32, isOutput=False)
        bqs[s] = nc.declare_dram_parameter(f"bqs_{s}", [HPC, 128], F32, isOutput=False)
        bk[s] = nc.declare_dram_parameter(f"bk_{s}", [HPC, 128], F32, isOutput=False)
        bks[s] = nc.declare_dram_parameter(f"bks_{s}", [HPC, 128], F32, isOutput=False)
        bv[s] = nc.declare_dram_parameter(f"bv_{s}", [HPC * 128], F32, isOutput=False)
        bo[s] = nc.declare_dram_parameter(f"bo_{s}", [KCH, 128], F32, isOutput=False)
        w1[s] = nc.declare_dram_parameter(f"w1_{s}", [JCH, 128, KCH, 128], FP8, isOutput=False)
        w3[s] = nc.declare_dram_parameter(f"w3_{s}", [JCH, 128, KCH, 128], FP8, isOutput=False)
        w2[s] = nc.declare_dram_parameter(f"w2_{s}", [KCH, 128, JCH, 128], FP8, isOutput=False)
        b1[s] = nc.declare_dram_parameter(f"b1_{s}", [JCH, 128], F32, isOutput=False)
        b3[s] = nc.declare_dram_parameter(f"b3_{s}", [JCH, 128], F32, isOutput=False)
        b2[s] = nc.declare_dram_parameter(f"b2_{s}", [KCH, 128], F32, isOutput=False)
        fnw[s] = nc.declare_dram_parameter(f"fnw_{s}", [KCH, 128], F32, isOutput=False)
    out_ext = nc.declare_dram_parameter("out", [2, D, TPC], F32, isOutput=True)

    # ---------------- internal DRAM ----------------
    rs_in = nc.dram_tensor("rs_in", [2, TPC], F32)
    rs_out = nc.dram_tensor("rs_out", [2 * NC, TPC], F32, addr_space="Shared")
    a2a_in = {s: nc.dram_tensor(f"a2a_in_{s}", [NC, HPC, 128, TPC], FP8)
              for s in ("x", "y")}
    a2a_out = {s: nc.dram_tensor(f"a2a_out_{s}", [NC, HPC, 128, TPC], FP8)
               for s in ("x", "y")}

    rg = [list(range(NC))]

    from contextlib import ExitStack
    with tile.TileContext(nc) as tc:
        with ExitStack() as es:
            const = es.enter_context(tc.tile_pool(name="const", bufs=1, side="left"))
            ones_bf = const.tile([128, 1], BF16)
            nc.vector.memset(ones_bf, 1.0)
            ones_f = const.tile([128, 1], F32)
            nc.vector.memset(ones_f, 1.0)
            ones_row = const.tile([1, 128], F32)
            nc.vector.memset(ones_row, 1.0)
            ones_row4 = const.tile([1, 128], F32, name="ones_row4", tag="ones_row4")
            nc.vector.memset(ones_row4, 4.0)
            ones_f8 = const.tile([128, 2, 16], FP8, name="ones_f8", tag="ones_f8")
            nc.vector.memset(ones_f8, 1.0)
            eps_sb = const.tile([128, 1], F32)
            nc.vector.memset(eps_sb, EPS)
            fnw_sb, bo_sb, b2_sb = {}, {}, {}
            bq_sb, bqs_sb, bk_sb, bks_sb, bv_sb = {}, {}, {}, {}, {}
            b1_sb, b3_sb = {}, {}
            for s in ("x", "y"):
                fnw_sb[s] = const.tile([128, KCH], F32, name=f"fnw{s}", tag=f"fnw{s}")
                nc.sync.dma_start(out=fnw_sb[s], in_=fnw[s].rearrange("k p -> p k"))
                bo_sb[s] = const.tile([128, KCH], F32, name=f"bo{s}", tag=f"bo{s}")
                nc.sync.dma_start(out=bo_sb[s], in_=bo[s].rearrange("k p -> p k"))
                b2_sb[s] = const.tile([128, KCH], F32, name=f"b2{s}", tag=f"b2{s}")
                nc.sync.dma_start(out=b2_sb[s], in_=b2[s].rearrange("k p -> p k"))
                b1_sb[s] = const.tile([128, JCH], F32, name=f"b1{s}", tag=f"b1{s}")
                nc.sync.dma_start(out=b1_sb[s], in_=b1[s].rearrange("k p -> p k"))
                b3_sb[s] = const.tile([128, JCH], F32, name=f"b3{s}", tag=f"b3{s}")
                nc.sync.dma_start(out=b3_sb[s], in_=b3[s].rearrange("k p -> p k"))
                bq_sb[s] = const.tile([128, HPC], F32, name=f"bq{s}", tag=f"bq{s}")
                nc.sync.dma_start(out=bq_sb[s], in_=bq[s].rearrange("h p -> p h"))
                bqs_sb[s] = const.tile([128, HPC], F32, name=f"bqs{s}", tag=f"bqs{s}")
                nc.sync.dma_start(out=bqs_sb[s], in_=bqs[s].rearrange("h p -> p h"))
                bk_sb[s] = const.tile([128, HPC], F32, name=f"bk{s}", tag=f"bk{s}")
                nc.sync.dma_start(out=bk_sb[s], in_=bk[s].rearrange("h p -> p h"))
                bks_sb[s] = const.tile([128, HPC], F32, name=f"bks{s}", tag=f"bks{s}")
                nc.sync.dma_start(out=bks_sb[s], in_=bks[s].rearrange("h p -> p h"))
                bv_sb[s] = const.tile([128, HPC * 128], F32, name=f"bv{s}", tag=f"bv{s}")
                nc.sync.dma_start(out=bv_sb[s],
                                  in_=bv[s][None, :].to_broadcast([128, HPC * 128]))

            for _rep in range(reps):
                aux_es = ExitStack()
                aux_ps = aux_es.enter_context(
                    tc.tile_pool(name="aux_ps", bufs=2, space="PSUM", side="left"))

                # ---------- phase 0: own-shard stats + tiny AllGathers ----------
                with tc.tile_pool(name="nrm", bufs=3, side="left") as nrm:
                    for si, s in enumerate(("x", "y")):
                        ms_ps = aux_ps.tile([1, TPC], F32, name="ms", tag="aux")
                        for kc in range(KCH):
                            t = nrm.tile([128, TPC], F32, name="xt", tag="xt")
                            nc.sync.dma_start(out=t, in_=xT[s][kc * 128:(kc + 1) * 128, :])
                            sq = nrm.tile([128, TPC], F32, name="sq", tag="sq")
                            nc.scalar.activation(out=sq, in_=t, func=AF.Square)
                            nc.tensor.matmul(ms_ps, ones_f, sq,
                                             start=(kc == 0), stop=(kc == KCH - 1))
                        sd = nrm.tile([1, TPC], F32, name="sd", tag="sd")
                        nc.scalar.activation(out=sd, in_=ms_ps, func=AF.Sqrt,
                                             bias=eps_sb[0:1, :], scale=1.0 / D)
                        rec = nrm.tile([1, TPC], F32, name="rec", tag="rec", bufs=2)
                        nc.vector.reciprocal(out=rec, in_=sd)
                        rec2 = nrm.tile([1, TPC], F32, name="rec2", tag="rec2", bufs=2)
                        nc.scalar.mul(out=rec2, in_=rec, mul=1.0 / (FSC * FSC))
                        nc.sync.dma_start(out=rs_in[si:si + 1, :], in_=rec2)
                if sim_local:
                    for g in range(NC):
                        nc.sync.dma_start(out=rs_out[2 * g:2 * g + 2, :], in_=rs_in[:])
                else:
                    nc.gpsimd.collective_compute(
                        "AllGather", AL.bypass, replica_groups=rg,
                        ins=[rs_in[:]], outs=[rs_out[:]])
                # column view rsc_sb[p, (c s a)] = rs_out[2c+s, a*128+p]
                rsc_sb0 = const.tile([128, 2 * NC * NA], F32, name="rsc0", tag="rsc0")
                nc.sync.dma_start(
                    out=rsc_sb0,
                    in_=rs_out.rearrange("(c s) (a p) -> p (c s a)", s=2, p=128))
                # rs_out rows carry rs/FSC^2; V path needs rs/FSC
                rsc_sb = const.tile([128, 2 * NC * NA], F32, name="rsc", tag="rsc")
                nc.scalar.mul(out=rsc_sb, in_=rsc_sb0, mul=FSC)

                # FFN-input tiles (outlive attention/Wo phases); right stack
                ffn_h = aux_es.enter_context(
                    tc.tile_pool(name="ffn_h", bufs=1, side="right"))
                h8 = {s: [ffn_h.tile([128, 2, TPC], FP8, name=f"h8{s}_{i}",
                                     tag=f"h8{s}_{i}") for i in range(KCH // 2)]
                      for s in ("x", "y")}
                ffn_h.seal()

                # ---------- persistent per-side QKV output tiles ----------
                qkt_cm, vp_cm = {}, {}
                qkt, vp = {}, {}
                qt_sb, kt_sb, v_sb = {}, {}, {}
                for s in ("y", "x"):
                    qkt_cm[s] = tc.tile_pool(name=f"qkt{s}", bufs=1, side="right")
                    qkt[s] = qkt_cm[s].__enter__()
                    vp_cm[s] = tc.tile_pool(name=f"vp{s}", bufs=1, side="right")
                    vp[s] = vp_cm[s].__enter__()
                    for h in range(HPC):
                        qt_sb[(s, h)] = qkt[s].tile([128, T], BF16,
                                                    name=f"qt{s}{h}", tag=f"qt{s}{h}")
                        kt_sb[(s, h)] = qkt[s].tile([128, T], BF16,
                                                    name=f"kt{s}{h}", tag=f"kt{s}{h}")
                    v_sb[s] = vp[s].tile([128, T // 256, 2, HPC * 128], FP8,
                                         name=f"v{s}", tag=f"v{s}")

                qkv_es = ExitStack()
                qkvw = qkv_es.enter_context(tc.tile_pool(name="qkvw", bufs=1, side="right"))
                wq_sb, wk_sb, wv_sb = {}, {}, {}
                for s in ("x", "y"):
                    for h in range(HPC):
                        wq_sb[(s, h)] = qkvw.tile([128, KCH // 2, 2, 128], FP8, name=f"wq{s}{h}", tag=f"wq{s}{h}")
                        nc.sync.dma_start(out=wq_sb[(s, h)], in_=wq[s][h])
                        wk_sb[(s, h)] = qkvw.tile([128, KCH // 2, 2, 128], FP8, name=f"wk{s}{h}", tag=f"wk{s}{h}")
                        nc.sync.dma_start(out=wk_sb[(s, h)], in_=wk[s][h])
                    wv_sb[s] = qkvw.tile([128, KCH // 2, 2, HPC * 128], FP8, name=f"wv{s}", tag=f"wv{s}")
                    nc.sync.dma_start(out=wv_sb[s], in_=wv[s][:])

                acts = qkv_es.enter_context(tc.tile_pool(name="acts", bufs=3, side="right"))
                ropet = qkv_es.enter_context(tc.tile_pool(name="ropet", bufs=2, side="right"))
                ropes = qkv_es.enter_context(tc.tile_pool(name="ropes", bufs=2, side="right"))
                qkv_ps = qkv_es.enter_context(
                    tc.tile_pool(name="qkv_ps", bufs=2, space="PSUM", side="right"))

                def emit_qkv_side(s):
                    so = "y" if s == "x" else "x"
                    si_s = 0 if s == "x" else 1
                    si_o = 1 - si_s
                    for qb in range(NQB):
                        pos = (qb * QBLK) % S
                        a_q = acts.tile([128, KCH // 2, 2, QBLK], FP8, name="aq", tag="a")
                        nc.sync.dma_start(
                            out=a_q, in_=xTf[so][:, :, :, qb * QBLK:(qb + 1) * QBLK])
                        a_kv = acts.tile([128, KCH // 2, 2, QBLK], FP8, name="akv", tag="a")
                        nc.sync.dma_start(
                            out=a_kv, in_=xTf[s][:, :, :, qb * QBLK:(qb + 1) * QBLK])
                        tb = {}
                        for n in ("cs_q", "sn_q", "cs_k", "sn_k"):
                            tt = ropet.tile([128, QBLK], BF16, tag=n)
                            nc.sync.dma_start(out=tt, in_=tabs[n][:, pos:pos + QBLK])
                            tb[n] = tt
                        rb = {}
                        for srci in (si_s, si_o):
                            row = 2 * (qb // 2) + srci
                            col = (qb % 2) * QBLK
                            rbt = ropes.tile([128, QBLK], F32, name=f"rb{srci}",
                                             tag=f"rb{srci}", bufs=1)
                            nc.sync.dma_start(
                                out=rbt,
                                in_=rs_out[row:row + 1,
                                           col:col + QBLK].to_broadcast(
                                               [128, QBLK]))
                            rb[srci] = rbt
                        for h in range(HPC):
                            for proj, wsb, bsb, bssb, cs_t, sn_t, dst, rbt in (
                                ("q", wq_sb[(s, h)], bq_sb[s], bqs_sb[s],
                                 tb["cs_q"], tb["sn_q"], qt_sb[(s, h)], rb[si_o]),
                                ("k", wk_sb[(s, h)], bk_sb[s], bks_sb[s],
                                 tb["cs_k"], tb["sn_k"], kt_sb[(s, h)], rb[si_s]),
                            ):
                                src_a = a_q if proj == "q" else a_kv
                                ps = qkv_ps.tile([128, QBLK], F32, name="qk", tag="qv")
                                for kp in range(KCH // 2):
                                    nc.tensor.matmul(
                                        ps, wsb[:, kp, :, :], src_a[:, kp, :, :],
                                        start=(kp == 0), stop=(kp == KCH // 2 - 1),
                                        perf_mode=mybir.MatmulPerfMode.DoubleRow)
                                # dst = (ps*rs + b)*cs + (swap(ps*rs) + bs)*sn
                                qs = ropes.tile([128, QBLK], F32, name="qs", tag="qs")
                                nc.vector.tensor_mul(qs, ps, rbt)
                                qsw = ropes.tile([128, QBLK], F32, name="qsw", tag="qsw")
                                nc.sync.dma_start(out=qsw[0:64, :], in_=qs[64:128, :])
                                nc.sync.dma_start(out=qsw[64:128, :], in_=qs[0:64, :])
                                t1 = ropes.tile([128, QBLK], F32, name="t1", tag="t1")
                                nc.vector.scalar_tensor_tensor(
                                    out=t1, in0=qs, scalar=bsb[:, h:h + 1],
                                    in1=cs_t, op0=AL.add, op1=AL.mult)
                                t2 = ropes.tile([128, QBLK], F32, name="t2", tag="t2")
                                nc.vector.scalar_tensor_tensor(
                                    out=t2, in0=qsw, scalar=bssb[:, h:h + 1],
                                    in1=sn_t, op0=AL.add, op1=AL.mult)
                                nc.vector.tensor_add(
                                    dst[:, qb * QBLK:(qb + 1) * QBLK], t1, t2)
                        # V in natural [token, hd] layout; rs per-partition here
                        for tk in range(QBLK // 128):
                            ci = ((qb // 2) * 2 + si_s) * NA + 2 * (qb % 2) + tk
                            tg = qb * 2 + tk
                            vps = qkv_ps.tile([128, HPC * 128], F32, name="v", tag="qv")
                            for kp in range(KCH // 2):
                                nc.tensor.matmul(
                                    vps, a_kv[:, kp, :, tk * 128:(tk + 1) * 128],
                                    wv_sb[s][:, kp, :, :],
                                    start=(kp == 0), stop=(kp == KCH // 2 - 1),
                                    perf_mode=mybir.MatmulPerfMode.DoubleRow)
                            nc.vector.scalar_tensor_tensor(
                                out=v_sb[s][:, tg // 2, tg % 2, :], in0=vps,
                                scalar=rsc_sb[:, ci:ci + 1],
                                in1=bv_sb[s], op0=AL.mult, op1=AL.add)

                def emit_attn_side(s, att_pt, att_sb, att_ps, att_po):
                    for b in range(B):
                        for gq in range(S // 512):
                            tq0 = b * S + gq * 512
                            g = tq0 // 512
                            for h in range(HPC):
                                po = att_po.tile([128, 512], F32, name="po", tag="po")
                                dn = aux_ps.tile([1, 512], F32, name="dn", tag="aux")
                                DRm = mybir.MatmulPerfMode.DoubleRow
                                for j in range(8):
                                    p8 = att_pt.tile([128, 2, 512], FP8, name="pt",
                                                     tag="pt")
                                    for i in (0, 1):
                                        sps = att_ps.tile([128, 512], F32,
                                                          name="s", tag="s")
                                        tkc = 2 * j + i
                                        nc.tensor.matmul(
                                            sps,
                                            kt_sb[(s, h)][:, b * S + tkc * 128:
                                                          b * S + (tkc + 1) * 128],
                                            qt_sb[(s, h)][:, tq0:tq0 + 512],
                                            start=True, stop=True)
                                        nc.scalar.activation(out=p8[:, i, :],
                                                             in_=sps, func=AF.Exp)
                                    nc.tensor.matmul(
                                        po,
                                        v_sb[s][:, b * (S // 256) + j, :,
                                                h * 128:(h + 1) * 128],
                                        p8, start=(j == 0), stop=(j == 7),
                                        perf_mode=DRm)
                                    nc.tensor.matmul(
                                        dn, ones_f8[:, :, 0:1], p8,
                                        start=(j == 0), stop=(j == 7),
                                        perf_mode=DRm)
                                rec = att_sb.tile([1, 512], F32, name="rec", tag="rec", bufs=1)
                                nc.vector.reciprocal(out=rec, in_=dn)
                                rbb = aux_ps.tile([128, 512], F32, name="rbb", tag="aux")
                                nc.tensor.matmul(rbb, ones_row4, rec,
                                                 start=True, stop=True)
                                rbs = att_sb.tile([128, 512], BF16, name="rbs",
                                                  tag="rbs", bufs=1)
                                nc.vector.tensor_copy(out=rbs, in_=rbb)
                                ot = att_sb.tile([128, 512], FP8, name="ot",
                                                 tag="ot", bufs=2)
                                nc.vector.tensor_mul(ot, po, rbs)
                                nc.sync.dma_start(out=a2a_in[s][g, h], in_=ot)
                    if sim_local:
                        nc.sync.dma_start(out=a2a_out[s][:], in_=a2a_in[s][:])
                    else:
                        nc.gpsimd.collective_compute(
                            "AllToAll", AL.bypass, replica_groups=rg,
                            ins=[a2a_in[s][:]], outs=[a2a_out[s][:]])

                def emit_wo_side(s, wo_w, wo_ps, ffn_h, h8):
                    o_sb = wo_w.tile([128, FC // 2, 2, TPC], FP8,
                                     name=f"o{s}", tag=f"o{s}")
                    nc.sync.dma_start(out=o_sb,
                                      in_=a2a_out[s].rearrange("c h p t -> p c h t"))
                    for kc in range(KCH):
                        wot = wo_w.tile([128, FC // 2, 2, 128], FP8, name="wot",
                                        tag="wot", bufs=2)
                        nc.sync.dma_start(out=wot, in_=wo[s][kc])
                        wps = wo_ps.tile([128, TPC], F32, name="wps", tag="wps")
                        for j in range(FC // 2):
                            nc.tensor.matmul(wps, wot[:, j, :, :], o_sb[:, j, :, :],
                                             start=(j == 0), stop=(j == FC // 2 - 1),
                                             perf_mode=mybir.MatmulPerfMode.DoubleRow)
                        nc.vector.tensor_scalar(
                            h8[s][kc // 2][:, kc % 2, :], wps,
                            bo_sb[s][:, kc:kc + 1], 1.0 / 64.0,
                            op0=AL.add, op1=AL.mult)

                # ---- emission ----
                emit_qkv_side("x")

                att_es = ExitStack()
                att_pt = att_es.enter_context(tc.tile_pool(name="att_pt", bufs=4, side="left"))
                att_sb = att_es.enter_context(tc.tile_pool(name="att_sb", bufs=2, side="left"))
                att_ps = att_es.enter_context(
                    tc.tile_pool(name="att_ps", bufs=2, space="PSUM", side="left"))
                att_po = att_es.enter_context(
                    tc.tile_pool(name="att_po", bufs=2, space="PSUM", side="left"))

                emit_attn_side("x", att_pt, att_sb, att_ps, att_po)
                emit_qkv_side("y")
                qkv_es.close()
                vp_cm["x"].__exit__(None, None, None)
                qkt_cm["x"].__exit__(None, None, None)

                wo_es = ExitStack()
                wo_w = wo_es.enter_context(tc.tile_pool(name="wo_w", bufs=1, side="right"))
                wo_ps = wo_es.enter_context(
                    tc.tile_pool(name="wo_ps", bufs=2, space="PSUM", side="right"))
                emit_wo_side("x", wo_w, wo_ps, ffn_h, h8)
                emit_attn_side("y", att_pt, att_sb, att_ps, att_po)
                emit_wo_side("y", wo_w, wo_ps, ffn_h, h8)
                att_es.close()
                wo_es.close()
                vp_cm["y"].__exit__(None, None, None)
                qkt_cm["y"].__exit__(None, None, None)

                # ---------- FFN + residual + final norm per side ----------
                with tc.tile_pool(name="ffn_g", bufs=1, side="left") as ffn_g, \
                     tc.tile_pool(name="ffn_w", bufs=3, side="left") as ffn_w, \
                     tc.tile_pool(name="ffn_t", bufs=2, side="left") as ffn_t, \
                     tc.tile_pool(name="ffn_fr", bufs=1, side="left") as ffn_fr, \
                     tc.tile_pool(name="ffn_ps", bufs=2, space="PSUM", side="left") as ffn_ps:
                    DR = mybir.MatmulPerfMode.DoubleRow
                    for s in ("x", "y"):
                        g_sb = []
                        for jc in range(JCH):
                            w1t = ffn_w.tile([128, KCH // 2, 2, 128], FP8,
                                             name="w1", tag="w1")
                            nc.sync.dma_start(out=w1t, in_=w1[s][jc])
                            w3t = ffn_w.tile([128, KCH // 2, 2, 128], FP8,
                                             name="w3", tag="w3")
                            nc.sync.dma_start(out=w3t, in_=w3[s][jc])
                            z1 = ffn_ps.tile([128, TPC], F32, name="z1", tag="z", bufs=3)
                            z3 = ffn_ps.tile([128, TPC], F32, name="z3", tag="z", bufs=3)
                            for kp in range(KCH // 2):
                                nc.tensor.matmul(z1, w1t[:, kp, :, :], h8[s][kp],
                                                 start=(kp == 0),
                                                 stop=(kp == KCH // 2 - 1),
                                                 perf_mode=DR)
                            for kp in range(KCH // 2):
                                nc.tensor.matmul(z3, w3t[:, kp, :, :], h8[s][kp],
                                                 start=(kp == 0),
                                                 stop=(kp == KCH // 2 - 1),
                                                 perf_mode=DR)
                            sz = ffn_t.tile([128, TPC], F32, name="sz", tag="sz")
                            nc.scalar.activation(out=sz, in_=z1, func=AF.Silu,
                                                 bias=b1_sb[s][:, jc:jc + 1],
                                                 scale=1.0 / (FSC * FSC))
                            gt = ffn_g.tile([128, TPC], BF16, name="gt",
                                            tag="gt", bufs=2)
                            nc.vector.scalar_tensor_tensor(
                                out=gt, in0=z3, scalar=b3_sb[s][:, jc:jc + 1], in1=sz,
                                op0=AL.add, op1=AL.mult)
                            if jc % 2 == 0:
                                g8 = ffn_g.tile([128, 2, TPC], FP8,
                                                name=f"g8{s}_{jc // 2}",
                                                tag=f"g8{s}_{jc // 2}")
                                g_sb.append(g8)
                            nc.scalar.mul(out=g_sb[jc // 2][:, jc % 2, :],
                                          in_=gt, mul=1.0 / FSC)
                        # W2 pass + residual + stats
                        ffr = []
                        ms_ps = aux_ps.tile([1, TPC], F32, name="ms2", tag="aux")
                        for kc in range(KCH):
                            w2t = ffn_w.tile([128, JCH // 2, 2, 128], FP8,
                                             name="w2", tag="w2", bufs=2)
                            nc.sync.dma_start(out=w2t, in_=w2[s][kc])
                            ff = ffn_ps.tile([128, TPC], F32, name="ff", tag="ff")
                            for jp in range(JCH // 2):
                                nc.tensor.matmul(ff, w2t[:, jp, :, :], g_sb[jp],
                                                 start=(jp == 0),
                                                 stop=(jp == JCH // 2 - 1),
                                                 perf_mode=DR)
                            xr = ffn_t.tile([128, TPC], F32, name="xr", tag="xr")
                            nc.sync.dma_start(out=xr,
                                              in_=xT[s][kc * 128:(kc + 1) * 128, :])
                            fr = ffn_fr.tile([128, TPC], F32, name=f"fr{s}{kc}",
                                             tag=f"fr{s}{kc}")
                            xr2 = ffn_t.tile([128, TPC], F32, name="xr2", tag="xr2")
                            nc.scalar.add(xr2, xr, b2_sb[s][:, kc:kc + 1])
                            nc.vector.scalar_tensor_tensor(
                                out=fr, in0=ff, scalar=1.0 / (FSC * FSC), in1=xr2,
                                op0=AL.mult, op1=AL.add)
                            ffr.append(fr)
                            sq = ffn_t.tile([128, TPC], F32, name="fsq", tag="fsq")
                            nc.scalar.activation(out=sq, in_=fr, func=AF.Square)
                            nc.tensor.matmul(ms_ps, ones_f, sq,
                                             start=(kc == 0), stop=(kc == KCH - 1))
                        sd = ffn_t.tile([1, TPC], F32, name="fsd", tag="fsd")
                        nc.scalar.activation(out=sd, in_=ms_ps, func=AF.Sqrt,
                                             bias=eps_sb[0:1, :], scale=1.0 / D)
                        rec = ffn_t.tile([1, TPC], F32, name="frec", tag="frec")
                        nc.vector.reciprocal(out=rec, in_=sd)
                        rb_ps = aux_ps.tile([128, TPC], F32, name="frb", tag="aux")
                        nc.tensor.matmul(rb_ps, ones_row, rec, start=True, stop=True)
                        rb = ffn_t.tile([128, TPC], F32, name="frbs", tag="frbs")
                        nc.scalar.copy(out=rb, in_=rb_ps)
                        si = 0 if s == "x" else 1
                        for kc in range(KCH):
                            ot = ffn_t.tile([128, TPC], F32, name="oto", tag="oto")
                            nc.vector.scalar_tensor_tensor(
                                out=ot, in0=ffr[kc], scalar=fnw_sb[s][:, kc:kc + 1],
                                in1=rb, op0=AL.mult, op1=AL.mult)
                            nc.sync.dma_start(
                                out=out_ext[si, kc * 128:(kc + 1) * 128, :], in_=ot)
                aux_es.close()

    nc.compile()
    return nc


def prepare_in_maps(inputs):
    perm = _rope_perm()
    x = np.asarray(inputs["x"], np.float32).reshape(T, D)
    y = np.asarray(inputs["y"], np.float32).reshape(T, D)
    anw = np.asarray(inputs["attn_norm_w"], np.float32)
    cos = np.asarray(inputs["freqs_cos"], np.float32).T  # [64, S]
    sin = np.asarray(inputs["freqs_sin"], np.float32).T
    cs = np.concatenate([cos, cos], 0)                   # [128, S]
    sn = np.concatenate([-sin, sin], 0)
    sc = 1.0 / math.sqrt(HD)

    common = {
        "cs_q": (cs * sc).astype(BF), "sn_q": (sn * sc).astype(BF),
        "cs_k": cs.astype(BF), "sn_k": sn.astype(BF),
    }
    # replicated raw activations, fp8 x FSC, pair-major [128, KCH//2, 2, T]
    for name, arr in (("xTf", x), ("yTf", y)):
        common[name] = np.ascontiguousarray(
            (arr.T * FSC).reshape(KCH // 2, 2, 128, T)
            .transpose(2, 0, 1, 3)).astype(E4)

    def tile_f8(w):
        K_, M_ = w.shape
        return np.ascontiguousarray(
            (w * FSC).reshape(K_ // 128, 128, M_ // 128, 128)
            .transpose(2, 1, 0, 3)).astype(E4)

    percore = [dict() for _ in range(NC)]
    for s in ("x", "y"):
        common[f"w1_{s}"] = tile_f8(np.asarray(inputs[f"W1_{s}"], np.float32))
        common[f"w3_{s}"] = tile_f8(np.asarray(inputs[f"W3_{s}"], np.float32))
        common[f"w2_{s}"] = tile_f8(np.asarray(inputs[f"W2_{s}"], np.float32))
        common[f"b1_{s}"] = np.asarray(inputs[f"b1_{s}"], np.float32).reshape(JCH, 128)
        common[f"b3_{s}"] = (np.asarray(inputs[f"b3_{s}"], np.float32)
                             .reshape(JCH, 128) * (FSC * FSC))
        common[f"b2_{s}"] = np.asarray(inputs[f"b2_{s}"], np.float32).reshape(KCH, 128)
        common[f"bo_{s}"] = (np.asarray(inputs[f"bo_{s}"], np.float32)
                             .reshape(KCH, 128) * (FSC * 64.0))
        common[f"fnw_{s}"] = np.asarray(
            inputs[f"ffn_norm_w_{s}"], np.float32).reshape(KCH, 128)
        Wo = np.asarray(inputs[f"Wo_{s}"], np.float32)
        common[f"wo_{s}"] = np.ascontiguousarray(
            (Wo * FSC).reshape(FC // 2, 2, 128, KCH, 128)
            .transpose(3, 2, 0, 1, 4)).astype(E4)
        Wq = np.asarray(inputs[f"Wq_{s}"], np.float32) * anw[:, None]
        Wk = np.asarray(inputs[f"Wk_{s}"], np.float32) * anw[:, None]
        Wv = np.asarray(inputs[f"Wv_{s}"], np.float32) * anw[:, None]
        bqv = np.asarray(inputs[f"bq_{s}"], np.float32)
        bkv = np.asarray(inputs[f"bk_{s}"], np.float32)
        bvv = np.asarray(inputs[f"bv_{s}"], np.float32)

        def tile_q(w):
            return np.ascontiguousarray(
                (w * FSC).reshape(KCH // 2, 2, 128, 128)
                .transpose(2, 0, 1, 3)).astype(E4)

        for c in range(NC):
            hsl = [HPC * c + h for h in range(HPC)]
            m = percore[c]
            m[f"wq_{s}"] = np.stack(
                [tile_q(Wq[:, h * HD:(h + 1) * HD][:, perm]) for h in hsl])
            m[f"wk_{s}"] = np.stack(
                [tile_q(Wk[:, h * HD:(h + 1) * HD][:, perm]) for h in hsl])
            vcols = np.concatenate([Wv[:, h * HD:(h + 1) * HD] for h in hsl], 1)
            m[f"wv_{s}"] = np.ascontiguousarray(
                (vcols * FSC).reshape(KCH // 2, 2, 128, HPC * 128)
                .transpose(2, 0, 1, 3)).astype(E4)
            bq_p = np.stack([bqv[h * HD:(h + 1) * HD][perm] for h in hsl])
            bk_p = np.stack([bkv[h * HD:(h + 1) * HD][perm] for h in hsl])
            m[f"bq_{s}"] = bq_p
            m[f"bqs_{s}"] = np.concatenate([bq_p[:, 64:], bq_p[:, :64]], 1)
            m[f"bk_{s}"] = bk_p
            m[f"bks_{s}"] = np.concatenate([bk_p[:, 64:], bk_p[:, :64]], 1)
            m[f"bv_{s}"] = np.concatenate(
                [bvv[h * HD:(h + 1) * HD] for h in hsl]) * FSC

    in_maps = []
    for c in range(NC):
        m = dict(common)
        m.update(percore[c])
        m["xT"] = np.ascontiguousarray(x[c * TPC:(c + 1) * TPC].T)
        m["yT"] = np.ascontiguousarray(y[c * TPC:(c + 1) * TPC].T)
        in_maps.append(m)
    return in_maps


def get_nc():
    if "nc" not in _CACHE:
        _CACHE["nc"] = build_nc()
    return _CACHE["nc"]


def kernel(**inputs):
    nc = get_nc()
    in_maps = prepare_in_maps(inputs)
    res = run_bass_kernel_spmd(nc, in_maps, core_ids=list(range(NC)))
    outs = []
    for si in range(2):
        full = np.concatenate([r["out"][si] for r in res.results], axis=1)  # [D, T]
        outs.append(np.ascontiguousarray(full.T).reshape(B, S, D))
    return outs[0], outs[1]


if __name__ == "__main__":
    nc = get_nc()
    print("build + compile OK")
